# revision 2
# baseline (speedup 1.0000x reference)
"""Causal self-attention (B=4, T=2048, C=1024, H=16) on 8 TRN2 NeuronCores.

Sharding: core = 2*b + g (b = batch 0..3, g = head-group 0..1). Each core
computes qkv + attention for its batch and its 8 heads, then a PARTIAL
output projection over its own 512 y-dims for ALL 1024 output columns.
The host adds the two partials per batch (partial-sum unsharding) - no
device collectives at all.

All matmuls run in fp16 (1 PE cycle/row, no min-width constraint),
accumulation in fp32 PSUM. Attention PV uses the cheap orientation
out[q,65] = ex^T @ [V | 1] (65 rows per (k-tile, q-subtile) instead of
512), with the softmax denominator from the ones column; y is normalized
with a per-partition scalar multiply and transposed back to [dims, T]
with the DMA XBAR transpose (off the PE).

The attention inner loop is Activation-engine paced (exp); qkv-proj and
out-proj matmul "filler" work is interleaved between score/PV groups via
a debt counter so the PE never idles waiting for exp.
"""
import collections
import numpy as np

D_MODEL = 1024
N_HEAD = 16
D_HEAD = 64
B = 4
T = 2048
N_CORES = 8
P = 128
PAIRS = 4          # head pairs per core
NQ = 4             # q-chunks of 512
QC = 512           # q chunk width
KT = D_MODEL // P  # 8 contraction tiles for the qkv projection
W_COLS = 1536      # 1024 q/k cols + 512 v cols per core

_RUNNER_CACHE = {}

# cost-model-ish estimates (ns) for the act-debt interleaver
_ACT_PER_ELEM = 0.833
_ACT_PER_OP = 185.0
_PE_PER_ROW = 0.4167


def _build(has_qk_bias: bool, _nphases: int = 5):
    from concourse import bacc
    import concourse.mybir as mybir
    from concourse.tile import TileContext
    from concourse.bass import ts

    f32 = mybir.dt.float32
    f16 = mybir.dt.float16
    KD = D_MODEL + (1 if has_qk_bias else 0)

    nc = bacc.Bacc("TRN2", target_bir_lowering=False, debug=False,
                   num_devices=N_CORES)
    xT = nc.dram_tensor("xT", [KD, T], f16, kind="ExternalInput")
    wa = nc.dram_tensor("wa", [KD, W_COLS], f16, kind="ExternalInput")
    wp = nc.dram_tensor("wp", [512, 1024], f16, kind="ExternalInput")
    tri = nc.dram_tensor("tri", [P, P], f16, kind="ExternalInput")
    out = nc.dram_tensor("out", [T, 1024], f16, kind="ExternalOutput")

    EXPF = mybir.ActivationFunctionType.Exp

    with TileContext(nc) as tc:
        with (
            tc.tile_pool(name="wts", bufs=1) as wts,
            tc.tile_pool(name="qk_res", bufs=1) as qk_res,
            tc.tile_pool(name="v_res", bufs=1) as v_res,
            tc.tile_pool(name="yt_res", bufs=1) as yt_res,
            tc.tile_pool(name="xs", bufs=2) as xs_pool,
            tc.tile_pool(name="exp", bufs=14) as ex_pool,
            tc.tile_pool(name="ysb", bufs=8) as ysb_pool,
            tc.tile_pool(name="rcp", bufs=4) as r_pool,
            tc.tile_pool(name="osb", bufs=4) as o_pool,
            tc.tile_pool(name="lin", bufs=2, space="PSUM") as lin_pool,
            tc.tile_pool(name="st", bufs=2, space="PSUM") as st_pool,
            tc.tile_pool(name="yps", bufs=2, space="PSUM") as y_pool,
        ):
            # ---------------- persistent tiles ----------------
            # wa_sb[i] holds contraction k-tiles 2i, 2i+1: [128, 2, 1536]
            wa_sb = [wts.tile([P, 2, W_COLS], f16, name=f"wa{i}")
                     for i in range(4)]
            wp_sb = [wts.tile([P, 1024], f16, name=f"wp{j}")
                     for j in range(PAIRS)]
            tri_sb = wts.tile([P, P], f16, name="tri_sb")
            if has_qk_bias:
                xrow = wts.tile([1, T], f16, name="xrow")
                wrow = wts.tile([1, W_COLS], f16, name="wrow")
            # qT[p][c]/kT[p][c]: [128 dims (2 heads x 64), 512 T]
            qT = [[qk_res.tile([P, QC], f16, name=f"qT{p}_{c}")
                   for c in range(NQ)] for p in range(PAIRS)]
            kT = [[qk_res.tile([P, QC], f16, name=f"kT{p}_{c}")
                   for c in range(NQ)] for p in range(PAIRS)]
            # v_sb[tt]: [128 keys, 8 heads, 65 (v | 1)]
            v_sb = [v_res.tile([P, 8, 65], f16, name=f"v{t}")
                    for t in range(T // P)]
            # yT[p][c]: [128 dims, 512 T] (normalized, transposed)
            yT = [[yt_res.tile([P, QC], f16, name=f"yT{p}_{c}")
                   for c in range(NQ)] for p in range(PAIRS)]

            # ---------------- DMA loads ----------------
            # x chunk tiles; chunk 0 split in 4 pieces for early start
            xt = [None] * NQ
            xt[0] = xs_pool.tile([P, KT, QC], f16, name="xt", tag="xt")

            def load_x_piece(i):
                nc.sync.dma_start(
                    out=xt[0][:, 2 * i:2 * i + 2, :],
                    in_=xT[ts(i, 2 * P), ts(0, QC)].rearrange(
                        "(j p) q -> p j q", p=P))

            def load_wa(i):
                nc.sync.dma_start(
                    out=wa_sb[i],
                    in_=wa[ts(i, 2 * P), :].rearrange("(j p) c -> p j c", p=P))

            # interleave so k-outer chunk-0 compute can start early
            load_wa(0)
            load_x_piece(0)
            load_wa(1)
            load_x_piece(1)
            load_wa(2)
            load_x_piece(2)
            load_wa(3)
            load_x_piece(3)
            for j in range(PAIRS):
                nc.sync.dma_start(out=wp_sb[j], in_=wp[ts(j, P), :])
            nc.sync.dma_start(out=tri_sb, in_=tri[:])
            if has_qk_bias:
                nc.sync.dma_start(out=xrow, in_=xT[D_MODEL:D_MODEL + 1, :])
                nc.sync.dma_start(out=wrow, in_=wa[D_MODEL:D_MODEL + 1, :])

            def load_x_chunk(n):
                xt[n] = xs_pool.tile([P, KT, QC], f16, name="xt", tag="xt")
                nc.sync.dma_start(
                    out=xt[n],
                    in_=xT[0:D_MODEL, ts(n, QC)].rearrange(
                        "(k p) q -> p k q", p=P))

            # ---------------- qkv projection helpers ----------------
            # w block col ranges: block 2p -> q pair p, 2p+1 -> k pair p,
            # block 8+j -> v (cols 1024 + j*128)
            def qk_copy_dest(blk, n):
                p, kind = divmod(blk, 2)
                return (qT if kind == 0 else kT)[p][n]

            def emit_qk_block(n, blk, ps):
                """Accumulate w-block x x-chunk into ps and copy to SBUF."""
                for k in range(KT):
                    nc.tensor.matmul(
                        ps[:], wa_sb[k // 2][:, k % 2, ts(blk, P)],
                        xt[n][:, k, :],
                        start=(k == 0),
                        stop=(k == KT - 1) and not has_qk_bias)
                if has_qk_bias:
                    nc.tensor.matmul(
                        ps[:], wrow[:, ts(blk, P)], xrow[:, ts(n, QC)],
                        start=False, stop=True)
                nc.vector.tensor_copy(out=qk_copy_dest(blk, n), in_=ps[:])

            def emit_v_block(n, tsub, ps):
                tt = 4 * n + tsub
                for k in range(KT):
                    nc.tensor.matmul(
                        ps[:], xt[n][:, k, ts(tsub, P)],
                        wa_sb[k // 2][:, k % 2, 1024:1536],
                        start=(k == 0),
                        stop=(k == KT - 1) and not has_qk_bias)
                if has_qk_bias:
                    nc.tensor.matmul(
                        ps[:], xrow[:, n * QC + tsub * P:
                                    n * QC + (tsub + 1) * P],
                        wrow[:, 1024:1536], start=False, stop=True)
                nc.vector.memset(v_sb[tt][:, :, 64:65], 1.0)
                nc.vector.tensor_copy(
                    out=v_sb[tt][:, :, 0:64],
                    in_=ps.rearrange("p (h c) -> p h c", c=64))

            def qkv_chunk_thunks(n):
                """Filler thunks for chunk n (needs xt[n] loaded)."""
                thunks = []
                for blk in range(8):
                    def t(blk=blk):
                        ps = lin_pool.tile([P, QC], f32, name="lps",
                                           tag="lps")
                        emit_qk_block(n, blk, ps)
                    thunks.append((t, 8 * QC * _PE_PER_ROW))
                for tsub in range(4):
                    def t(tsub=tsub):
                        ps = lin_pool.tile([P, QC], f32, name="lps",
                                           tag="lps")
                        emit_v_block(n, tsub, ps)
                    thunks.append((t, 8 * QC * _PE_PER_ROW))
                return thunks

            # ---------------- chunk 0: k-outer for early start ----------
            # 3 passes of 4 accumulators (2 lin tiles + 2 halves of an st
            # tile) so compute streams while wa/x DMAs land.
            for pass_blocks in ([0, 1, 2, 3], [4, 5, 6, 7],
                                [(0, 0), (0, 1), (0, 2), (0, 3)]):
                l0 = lin_pool.tile([P, QC], f32, name="lps", tag="lps")
                l1 = lin_pool.tile([P, QC], f32, name="lps", tag="lps")
                s0 = st_pool.tile([P, 2, QC], f32, name="st", tag="st")
                accs = [l0, l1, s0[:, 0, :], s0[:, 1, :]]
                is_v = isinstance(pass_blocks[0], tuple)
                for k in range(KT):
                    for a, blk in zip(accs, pass_blocks):
                        if is_v:
                            _, tsub = blk
                            nc.tensor.matmul(
                                a[:], xt[0][:, k, ts(tsub, P)],
                                wa_sb[k // 2][:, k % 2, 1024:1536],
                                start=(k == 0),
                                stop=(k == KT - 1) and not has_qk_bias)
                        else:
                            nc.tensor.matmul(
                                a[:], wa_sb[k // 2][:, k % 2, ts(blk, P)],
                                xt[0][:, k, :],
                                start=(k == 0),
                                stop=(k == KT - 1) and not has_qk_bias)
                for a, blk in zip(accs, pass_blocks):
                    if is_v:
                        _, tsub = blk
                        if has_qk_bias:
                            nc.tensor.matmul(
                                a[:], xrow[:, ts(tsub, P)],
                                wrow[:, 1024:1536], start=False, stop=True)
                        nc.vector.memset(v_sb[tsub][:, :, 64:65], 1.0)
                        nc.vector.tensor_copy(
                            out=v_sb[tsub][:, :, 0:64],
                            in_=a.rearrange("p (h c) -> p h c", c=64))
                    else:
                        if has_qk_bias:
                            nc.tensor.matmul(
                                a[:], wrow[:, ts(blk, P)], xrow[:, ts(0, QC)],
                                start=False, stop=True)
                        nc.vector.tensor_copy(out=qk_copy_dest(blk, 0),
                                              in_=a[:])

            # ---------------- attention ----------------
            def attn_chunk(c, pair_list, fillers, debt):
                """Attention for q-chunk c over pairs in pair_list.

                fillers: deque of (thunk, pe_ns); popped when the act-debt
                (exp time not covered by attention PE work) exceeds one
                thunk's worth, keeping the PE busy while Act catches up.
                """
                for p in pair_list:
                    ysub = [ysb_pool.tile([P, 2, 64], f16, name="ysb",
                                          tag="ysb") for _ in range(4)]
                    for h in (0, 1):
                        hb = h * 64
                        lh = 2 * p + h
                        y_ps = y_pool.tile([P, 4, P], f32, name="yps",
                                           tag="yps")
                        # groups: (kt_a, kt_b, so_a, so_b)
                        groups = [(2 * g, 2 * g + 1, 0, 0)
                                  for g in range(2 * c)]
                        groups.append((4 * c, 4 * c + 1, 0, P))
                        groups.append((4 * c + 2, 4 * c + 3, 2 * P, 3 * P))
                        pieces = []  # (kt, so, ex_tile, j)
                        for ka, kb, soa, sob in groups:
                            st = st_pool.tile([P, 2, QC], f32, name="st",
                                              tag="st")
                            ex = ex_pool.tile([P, 2, QC], f16, name="ex",
                                              tag="ex")
                            for j, (kt, so) in enumerate(((ka, soa),
                                                          (kb, sob))):
                                nc.tensor.matmul(
                                    st[:, j, so:QC],
                                    kT[p][kt // 4][hb:hb + 64,
                                                   ts(kt % 4, P)],
                                    qT[p][c][hb:hb + 64, so:QC],
                                    start=True, stop=True)
                                pieces.append((kt, so, ex, j))
                            # exp
                            if soa == sob:
                                nc.scalar.activation(ex[:], st[:], EXPF,
                                                     scale=0.125)
                                act_ns = 2 * QC * _ACT_PER_ELEM + _ACT_PER_OP
                            else:
                                act_ns = 0.0
                                for j, so in enumerate((soa, sob)):
                                    nc.scalar.activation(
                                        ex[:, j, so:QC], st[:, j, so:QC],
                                        EXPF, scale=0.125)
                                    act_ns += ((QC - so) * _ACT_PER_ELEM
                                               + _ACT_PER_OP)
                            # mask the 128-wide diagonal transition blocks
                            pe_ns = 0.0
                            for j, (kt, so) in enumerate(((ka, soa),
                                                          (kb, sob))):
                                if kt >= 4 * c:
                                    nc.vector.tensor_mul(
                                        ex[:, j, so:so + P],
                                        ex[:, j, so:so + P],
                                        tri_sb[:])
                                pe_ns += (QC - so) * _PE_PER_ROW
                            # act-debt bookkeeping + filler dispatch
                            debt[0] += act_ns - pe_ns
                            while fillers and debt[0] > fillers[0][1]:
                                t, tns = fillers.popleft()
                                t()
                                debt[0] -= tns
                        # PV: one sequential PSUM accumulation group per
                        # q-subtile (a PSUM bank allows only one open
                        # accumulation group at a time)
                        for qs in range(4):
                            last_kt = 4 * c + qs
                            for kt, so, ex, j in pieces:
                                if kt > last_kt:
                                    continue
                                jj = kt - 4 * c
                                if jj >= 0 and qs < jj:
                                    continue
                                nc.tensor.matmul(
                                    y_ps[:, qs, 0:65],
                                    ex[:, j, ts(qs, P)],
                                    v_sb[kt][:, lh, :],
                                    start=(kt == 0),
                                    stop=(kt == last_kt))
                                debt[0] -= 65 * _PE_PER_ROW
                            while fillers and debt[0] > fillers[0][1]:
                                t, tns = fillers.popleft()
                                t()
                                debt[0] -= tns
                        # normalize: recip of denominators, scale 64 v-dims
                        r = r_pool.tile([P, 4], f32, name="rcp", tag="rcp")
                        nc.vector.reciprocal(out=r[:],
                                             in_=y_ps[:, :, 64:65])
                        for qs in range(4):
                            nc.vector.tensor_scalar_mul(
                                out=ysub[qs][:, h, :],
                                in0=y_ps[:, qs, 0:64],
                                scalar1=r[:, qs:qs + 1])
                    # transpose y [128 q, 128 dims] -> yT [128 dims, 128 q]
                    for qs in range(4):
                        nc.sync.dma_start(out=yT[p][c][:, ts(qs, P)],
                                          in_=ysub[qs][:], transpose=True)

            def proj_thunks(c):
                """Partial out-proj for T-tiles of chunk c (all pairs)."""
                thunks = []
                osb = {}

                def mk(tt, half):
                    def t():
                        if half == 0:
                            osb[tt] = o_pool.tile([P, 1024], f16, name="osb",
                                                  tag="osb")
                        ps = lin_pool.tile([P, QC], f32, name="lps",
                                           tag="lps")
                        for p in range(PAIRS):
                            nc.tensor.matmul(
                                ps[:], yT[p][tt // 4][:, ts(tt % 4, P)],
                                wp_sb[p][:, ts(half, QC)],
                                start=(p == 0), stop=(p == PAIRS - 1))
                        nc.vector.tensor_copy(
                            out=osb[tt][:, ts(half, QC)], in_=ps[:])
                        if half == 1:
                            nc.sync.dma_start(out=out[ts(tt, P), :],
                                              in_=osb[tt][:])
                    return t

                for tt in range(4 * c, 4 * c + 4):
                    for half in (0, 1):
                        thunks.append((mk(tt, half),
                                       PAIRS * QC * _PE_PER_ROW))
                return thunks

            # ---------------- main schedule ----------------
            debt = [0.0]
            load_x_chunk(1)
            fillers = collections.deque(qkv_chunk_thunks(1))
            attn_chunk(0, range(PAIRS), fillers, debt)
            while fillers:
                fillers.popleft()[0]()

            load_x_chunk(2)
            fillers = collections.deque(qkv_chunk_thunks(2))
            attn_chunk(1, range(PAIRS), fillers, debt)
            while fillers:
                fillers.popleft()[0]()

            load_x_chunk(3)
            fillers = collections.deque(qkv_chunk_thunks(3))
            attn_chunk(2, range(PAIRS), fillers, debt)
            while fillers:
                fillers.popleft()[0]()

            fillers = collections.deque(
                proj_thunks(0) + proj_thunks(1) + proj_thunks(2))
            attn_chunk(3, range(PAIRS), fillers, debt)
            while fillers:
                fillers.popleft()[0]()

            for t, _ in proj_thunks(3):
                t()

    nc.compile()
    return nc


def _make_runner(nc):
    """Reusable 8-core SPMD runner (jit built once)."""
    import jax
    from jax.sharding import Mesh, PartitionSpec
    from jax.experimental.shard_map import shard_map
    from concourse import bass2jax
    import concourse.mybir as mybir

    bass2jax.install_neuronx_cc_hook()
    partition_name = (nc.partition_id_tensor.name
                      if nc.partition_id_tensor else None)
    in_names, out_names, out_avals, zero_outs = [], [], [], []
    for alloc in nc.m.functions[0].allocations:
        if not isinstance(alloc, mybir.MemoryLocationSet):
            continue
        name = alloc.memorylocations[0].name
        if alloc.kind == "ExternalInput":
            if name != partition_name:
                in_names.append(name)
        elif alloc.kind == "ExternalOutput":
            shape = tuple(alloc.tensor_shape)
            dtype = mybir.dt.np(alloc.dtype)
            out_names.append(name)
            out_avals.append(jax.core.ShapedArray(shape, dtype))
            zero_outs.append(np.zeros(shape, dtype))
    n_params = len(in_names)
    n_outs = len(out_avals)
    all_in = list(in_names) + list(out_names)
    if partition_name is not None:
        all_in.append(partition_name)

    def _body(*args):
        operands = list(args)
        if partition_name is not None:
            operands.append(bass2jax.partition_id_tensor())
        outs = bass2jax._bass_exec_p.bind(
            *operands,
            out_avals=tuple(out_avals),
            in_names=tuple(all_in),
            out_names=tuple(out_names),
            lowering_input_output_aliases=(),
            sim_require_finite=True,
            sim_require_nnan=True,
            nc=nc,
        )
        return tuple(outs)

    devices = jax.devices()[:N_CORES]
    mesh = Mesh(np.asarray(devices), ("core",))
    in_specs = (PartitionSpec("core"),) * (n_params + n_outs)
    out_specs = (PartitionSpec("core"),) * n_outs
    donate = tuple(range(n_params, n_params + n_outs))
    sharded = jax.jit(
        shard_map(_body, mesh=mesh, in_specs=in_specs, out_specs=out_specs,
                  check_rep=False),
        donate_argnums=donate, keep_unused=True)

    def run(in_maps):
        per_core = [[np.asarray(m[k]) for k in in_names] for m in in_maps]
        concat_in = [
            np.concatenate([per_core[c][i] for c in range(N_CORES)], axis=0)
            for i in range(n_params)]
        concat_zeros = [
            np.zeros((N_CORES * z.shape[0], *z.shape[1:]), z.dtype)
            for z in zero_outs]
        outs = sharded(*concat_in, *concat_zeros)
        jax.block_until_ready(outs)
        return [
            {name: np.asarray(outs[i]).reshape(N_CORES, *out_avals[i].shape)[c]
             for i, name in enumerate(out_names)}
            for c in range(N_CORES)]

    return run


def kernel(x, w_qkv, b_qkv, w_proj, b_proj):
    x = np.asarray(x, dtype=np.float32)
    w_qkv = np.asarray(w_qkv, dtype=np.float32)
    b_qkv = np.asarray(b_qkv, dtype=np.float32)
    w_proj = np.asarray(w_proj, dtype=np.float32)
    b_proj = np.asarray(b_proj, dtype=np.float32)

    w_q, w_k, w_v = w_qkv[0:1024], w_qkv[1024:2048], w_qkv[2048:3072]
    b_q, b_k, b_v = b_qkv[0:1024], b_qkv[1024:2048], b_qkv[2048:3072]
    has_qk_bias = bool(np.any(b_q) or np.any(b_k))

    key = ("runner", has_qk_bias)
    if key not in _RUNNER_CACHE:
        nc = _build(has_qk_bias)
        _RUNNER_CACHE[key] = _make_runner(nc)
    run = _RUNNER_CACHE[key]

    # causal transition-block mask: tri[k, i] = 1.0 iff k <= i
    kk = np.arange(P)
    tri = (kk[:, None] <= kk[None, :]).astype(np.float16)

    in_maps = []
    for core in range(N_CORES):
        b, g = divmod(core, 2)
        xT_c = x[b].T.astype(np.float16)  # [1024, 2048]
        if has_qk_bias:
            xT_c = np.concatenate(
                [xT_c, np.ones((1, T), np.float16)], axis=0)
        KD = D_MODEL + (1 if has_qk_bias else 0)
        wa_c = np.empty((KD, W_COLS), np.float32)
        wp_c = np.empty((512, 1024), np.float32)
        for p in range(PAIRS):
            hA = 8 * g + 2 * p
            hB = hA + 1
            cols = p * 256
            wa_c[:D_MODEL, cols + 0:cols + 64] = w_q[hA * 64:(hA + 1) * 64].T
            wa_c[:D_MODEL, cols + 64:cols + 128] = w_q[hB * 64:(hB + 1) * 64].T
            wa_c[:D_MODEL, cols + 128:cols + 192] = w_k[hA * 64:(hA + 1) * 64].T
            wa_c[:D_MODEL, cols + 192:cols + 256] = w_k[hB * 64:(hB + 1) * 64].T
            if has_qk_bias:
                wa_c[D_MODEL, cols + 0:cols + 64] = b_q[hA * 64:(hA + 1) * 64]
                wa_c[D_MODEL, cols + 64:cols + 128] = b_q[hB * 64:(hB + 1) * 64]
                wa_c[D_MODEL, cols + 128:cols + 192] = b_k[hA * 64:(hA + 1) * 64]
                wa_c[D_MODEL, cols + 192:cols + 256] = b_k[hB * 64:(hB + 1) * 64]
            # wp rows pair-major: [hA dims 0..63 | hB dims 64..127]
            wp_c[p * 128:p * 128 + 64, :] = w_proj.T[hA * 64:(hA + 1) * 64, :]
            wp_c[p * 128 + 64:p * 128 + 128, :] = \
                w_proj.T[hB * 64:(hB + 1) * 64, :]
        # v columns, head-major for the group
        for lh in range(8):
            head = 8 * g + lh
            wa_c[:D_MODEL, 1024 + lh * 64:1024 + (lh + 1) * 64] = \
                w_v[head * 64:(head + 1) * 64].T
            if has_qk_bias:
                wa_c[D_MODEL, 1024 + lh * 64:1024 + (lh + 1) * 64] = \
                    b_v[head * 64:(head + 1) * 64]
        in_maps.append({
            "xT": xT_c,
            "wa": wa_c.astype(np.float16),
            "wp": wp_c.astype(np.float16),
            "tri": tri,
        })

    results = run(in_maps)

    # partial-sum unshard: the two head-group cores of each batch each
    # produced out_partial[T, 1024]; add them.
    out = np.empty((B, T, D_MODEL), dtype=np.float32)
    for b in range(B):
        out[b] = (results[2 * b]["out"].astype(np.float32)
                  + results[2 * b + 1]["out"].astype(np.float32))

    # exact host-side bias folds (v-bias rides softmax row-sums == 1 and is
    # on-device in the qk-bias build; proj bias is additive)
    if np.any(b_v) and not has_qk_bias:
        out += (b_v @ w_proj.T)[None, None, :]
    if np.any(b_proj):
        out += b_proj[None, None, :]
    return out


# revision 3
# speedup vs baseline: 1.0164x; 1.0164x over previous
"""Causal self-attention (B=4, T=2048, C=1024, H=16) on 8 TRN2 NeuronCores.

Sharding: core = 2*b + g (b = batch 0..3, g = head-group 0..1). Each core
computes qkv + attention for its batch and its 8 heads, then a PARTIAL
output projection over its own 512 y-dims for ALL 1024 output columns.
The host adds the two partials per batch (partial-sum unsharding) - no
device collectives at all.

All matmuls run in fp16 (1 PE cycle/row, no min-width constraint),
accumulation in fp32 PSUM. Attention PV uses the cheap orientation
out[q,65] = ex^T @ [V | 1] (65 rows per (k-tile, q-subtile) instead of
512), with the softmax denominator from the ones column; y is normalized
with a per-partition scalar multiply and transposed back to [dims, T]
with the DMA XBAR transpose (off the PE).

The attention inner loop is Activation-engine paced (exp); qkv-proj and
out-proj matmul "filler" work is interleaved between score/PV groups via
a debt counter so the PE never idles waiting for exp.
"""
import collections
import numpy as np

D_MODEL = 1024
N_HEAD = 16
D_HEAD = 64
B = 4
T = 2048
N_CORES = 8
P = 128
PAIRS = 4          # head pairs per core
NQ = 4             # q-chunks of 512
QC = 512           # q chunk width
KT = D_MODEL // P  # 8 contraction tiles for the qkv projection
W_COLS = 1536      # 1024 q/k cols + 512 v cols per core

_RUNNER_CACHE = {}

# cost-model-ish estimates (ns) for the act-debt interleaver
_ACT_PER_ELEM = 0.833
_ACT_PER_OP = 185.0
_PE_PER_ROW = 0.4167


def _build(has_qk_bias: bool, _nphases: int = 5):
    from concourse import bacc
    import concourse.mybir as mybir
    from concourse.tile import TileContext
    from concourse.bass import ts

    f32 = mybir.dt.float32
    f16 = mybir.dt.float16
    KD = D_MODEL + (1 if has_qk_bias else 0)

    nc = bacc.Bacc("TRN2", target_bir_lowering=False, debug=False,
                   num_devices=N_CORES)
    xT = nc.dram_tensor("xT", [KD, T], f16, kind="ExternalInput")
    wa = nc.dram_tensor("wa", [KD, W_COLS], f16, kind="ExternalInput")
    wp = nc.dram_tensor("wp", [512, 1024], f16, kind="ExternalInput")
    tri = nc.dram_tensor("tri", [P, P], f16, kind="ExternalInput")
    out = nc.dram_tensor("out", [T, 1024], f16, kind="ExternalOutput")

    EXPF = mybir.ActivationFunctionType.Exp

    with TileContext(nc) as tc:
        with (
            tc.tile_pool(name="wts", bufs=1) as wts,
            tc.tile_pool(name="qk_res", bufs=1) as qk_res,
            tc.tile_pool(name="v_res", bufs=1) as v_res,
            tc.tile_pool(name="yt_res", bufs=1) as yt_res,
            tc.tile_pool(name="xs", bufs=2) as xs_pool,
            tc.tile_pool(name="exp", bufs=20) as ex_pool,
            tc.tile_pool(name="ysb", bufs=8) as ysb_pool,
            tc.tile_pool(name="rcp", bufs=4) as r_pool,
            tc.tile_pool(name="osb", bufs=4) as o_pool,
            tc.tile_pool(name="lin", bufs=2, space="PSUM") as lin_pool,
            tc.tile_pool(name="st", bufs=2, space="PSUM") as st_pool,
            tc.tile_pool(name="yps", bufs=2, space="PSUM") as y_pool,
        ):
            # ---------------- persistent tiles ----------------
            # wa_sb[i] holds contraction k-tiles 2i, 2i+1: [128, 2, 1536]
            wa_sb = [wts.tile([P, 2, W_COLS], f16, name=f"wa{i}")
                     for i in range(4)]
            wp_sb = [wts.tile([P, 1024], f16, name=f"wp{j}")
                     for j in range(PAIRS)]
            tri_sb = wts.tile([P, P], f16, name="tri_sb")
            if has_qk_bias:
                xrow = wts.tile([1, T], f16, name="xrow")
                wrow = wts.tile([1, W_COLS], f16, name="wrow")
            # qT[p][c]/kT[p][c]: [128 dims (2 heads x 64), 512 T]
            qT = [[qk_res.tile([P, QC], f16, name=f"qT{p}_{c}")
                   for c in range(NQ)] for p in range(PAIRS)]
            kT = [[qk_res.tile([P, QC], f16, name=f"kT{p}_{c}")
                   for c in range(NQ)] for p in range(PAIRS)]
            # v_sb[tt]: [128 keys, 8 heads, 65 (v | 1)]
            v_sb = [v_res.tile([P, 8, 65], f16, name=f"v{t}")
                    for t in range(T // P)]
            # yT[p][c]: [128 dims, 512 T] (normalized, transposed)
            yT = [[yt_res.tile([P, QC], f16, name=f"yT{p}_{c}")
                   for c in range(NQ)] for p in range(PAIRS)]

            # ---------------- DMA loads ----------------
            # x chunk tiles; chunk 0 split in 4 pieces for early start
            xt = [None] * NQ
            xt[0] = xs_pool.tile([P, KT, QC], f16, name="xt", tag="xt")

            def load_x_piece(i):
                nc.sync.dma_start(
                    out=xt[0][:, 2 * i:2 * i + 2, :],
                    in_=xT[ts(i, 2 * P), ts(0, QC)].rearrange(
                        "(j p) q -> p j q", p=P))

            def load_wa(i, j):
                nc.sync.dma_start(
                    out=wa_sb[i][:, j, :],
                    in_=wa[ts(2 * i + j, P), :])

            # interleave so k-outer chunk-0 compute can start early
            for i in range(4):
                load_wa(i, 0)
                load_x_piece(i)
                load_wa(i, 1)
            for j in range(PAIRS):
                nc.sync.dma_start(out=wp_sb[j], in_=wp[ts(j, P), :])
            nc.sync.dma_start(out=tri_sb, in_=tri[:])
            if has_qk_bias:
                nc.sync.dma_start(out=xrow, in_=xT[D_MODEL:D_MODEL + 1, :])
                nc.sync.dma_start(out=wrow, in_=wa[D_MODEL:D_MODEL + 1, :])

            def load_x_chunk(n):
                xt[n] = xs_pool.tile([P, KT, QC], f16, name="xt", tag="xt")
                nc.sync.dma_start(
                    out=xt[n],
                    in_=xT[0:D_MODEL, ts(n, QC)].rearrange(
                        "(k p) q -> p k q", p=P))

            # ---------------- qkv projection helpers ----------------
            # w block col ranges: block 2p -> q pair p, 2p+1 -> k pair p,
            # block 8+j -> v (cols 1024 + j*128)
            def qk_copy_dest(blk, n):
                p, kind = divmod(blk, 2)
                return (qT if kind == 0 else kT)[p][n]

            def emit_qk_block(n, blk, ps):
                """Accumulate w-block x x-chunk into ps and copy to SBUF."""
                for k in range(KT):
                    nc.tensor.matmul(
                        ps[:], wa_sb[k // 2][:, k % 2, ts(blk, P)],
                        xt[n][:, k, :],
                        start=(k == 0),
                        stop=(k == KT - 1) and not has_qk_bias)
                if has_qk_bias:
                    nc.tensor.matmul(
                        ps[:], wrow[:, ts(blk, P)], xrow[:, ts(n, QC)],
                        start=False, stop=True)
                nc.vector.tensor_copy(out=qk_copy_dest(blk, n), in_=ps[:])

            def emit_v_block(n, tsub, ps):
                tt = 4 * n + tsub
                for k in range(KT):
                    nc.tensor.matmul(
                        ps[:], xt[n][:, k, ts(tsub, P)],
                        wa_sb[k // 2][:, k % 2, 1024:1536],
                        start=(k == 0),
                        stop=(k == KT - 1) and not has_qk_bias)
                if has_qk_bias:
                    nc.tensor.matmul(
                        ps[:], xrow[:, n * QC + tsub * P:
                                    n * QC + (tsub + 1) * P],
                        wrow[:, 1024:1536], start=False, stop=True)
                nc.vector.memset(v_sb[tt][:, :, 64:65], 1.0)
                nc.vector.tensor_copy(
                    out=v_sb[tt][:, :, 0:64],
                    in_=ps.rearrange("p (h c) -> p h c", c=64))

            def qkv_chunk_thunks(n):
                """Filler thunks for chunk n (needs xt[n] loaded)."""
                thunks = []
                for blk in range(8):
                    def t(blk=blk):
                        ps = lin_pool.tile([P, QC], f32, name="lps",
                                           tag="lps")
                        emit_qk_block(n, blk, ps)
                    thunks.append((t, 8 * QC * _PE_PER_ROW))
                for tsub in range(4):
                    def t(tsub=tsub):
                        ps = lin_pool.tile([P, QC], f32, name="lps",
                                           tag="lps")
                        emit_v_block(n, tsub, ps)
                    thunks.append((t, 8 * QC * _PE_PER_ROW))
                return thunks

            # ---------------- chunk 0: k-outer for early start ----------
            # 3 passes of 4 accumulators (2 lin tiles + 2 halves of an st
            # tile) so compute streams while wa/x DMAs land.
            for pass_blocks in ([0, 1, 2, 3], [4, 5, 6, 7],
                                [(0, 0), (0, 1), (0, 2), (0, 3)]):
                l0 = lin_pool.tile([P, QC], f32, name="lps", tag="lps")
                l1 = lin_pool.tile([P, QC], f32, name="lps", tag="lps")
                s0 = st_pool.tile([P, 2, QC], f32, name="st", tag="st")
                accs = [l0, l1, s0[:, 0, :], s0[:, 1, :]]
                is_v = isinstance(pass_blocks[0], tuple)
                for k in range(KT):
                    for a, blk in zip(accs, pass_blocks):
                        if is_v:
                            _, tsub = blk
                            nc.tensor.matmul(
                                a[:], xt[0][:, k, ts(tsub, P)],
                                wa_sb[k // 2][:, k % 2, 1024:1536],
                                start=(k == 0),
                                stop=(k == KT - 1) and not has_qk_bias)
                        else:
                            nc.tensor.matmul(
                                a[:], wa_sb[k // 2][:, k % 2, ts(blk, P)],
                                xt[0][:, k, :],
                                start=(k == 0),
                                stop=(k == KT - 1) and not has_qk_bias)
                for a, blk in zip(accs, pass_blocks):
                    if is_v:
                        _, tsub = blk
                        if has_qk_bias:
                            nc.tensor.matmul(
                                a[:], xrow[:, ts(tsub, P)],
                                wrow[:, 1024:1536], start=False, stop=True)
                        nc.vector.memset(v_sb[tsub][:, :, 64:65], 1.0)
                        nc.vector.tensor_copy(
                            out=v_sb[tsub][:, :, 0:64],
                            in_=a.rearrange("p (h c) -> p h c", c=64))
                    else:
                        if has_qk_bias:
                            nc.tensor.matmul(
                                a[:], wrow[:, ts(blk, P)], xrow[:, ts(0, QC)],
                                start=False, stop=True)
                        nc.vector.tensor_copy(out=qk_copy_dest(blk, 0),
                                              in_=a[:])

            # ---------------- attention ----------------
            def attn_chunk(c, pair_list, fillers, debt):
                """Attention for q-chunk c over pairs in pair_list.

                fillers: deque of (thunk, pe_ns); popped when the act-debt
                (exp time not covered by attention PE work) exceeds one
                thunk's worth, keeping the PE busy while Act catches up.
                """
                # groups: (kt_a, kt_b, so_a, so_b)
                groups = [(2 * g, 2 * g + 1, 0, 0) for g in range(2 * c)]
                groups.append((4 * c, 4 * c + 1, 0, P))
                groups.append((4 * c + 2, 4 * c + 3, 2 * P, 3 * P))
                for p in pair_list:
                    ysub = [ysb_pool.tile([P, 2, 64], f16, name="ysb",
                                          tag="ysb") for _ in range(4)]
                    pieces = {0: [], 1: []}  # h -> (kt, so, ex_tile, j)
                    for h in (0, 1):
                        hb = h * 64
                        for ka, kb, soa, sob in groups:
                            st = st_pool.tile([P, 2, QC], f32, name="st",
                                              tag="st")
                            ex = ex_pool.tile([P, 2, QC], f16, name="ex",
                                              tag="ex")
                            for j, (kt, so) in enumerate(((ka, soa),
                                                          (kb, sob))):
                                nc.tensor.matmul(
                                    st[:, j, so:QC],
                                    kT[p][kt // 4][hb:hb + 64,
                                                   ts(kt % 4, P)],
                                    qT[p][c][hb:hb + 64, so:QC],
                                    start=True, stop=True)
                                pieces[h].append((kt, so, ex, j))
                            # exp: one op per group over [min_so:512] of
                            # both pieces; the [soa:sob) part of piece b is
                            # unwritten PSUM whose exp is never read by PV
                            mso = min(soa, sob)
                            nc.scalar.activation(ex[:, :, mso:QC],
                                                 st[:, :, mso:QC], EXPF,
                                                 scale=0.125)
                            act_ns = (2 * (QC - mso) * _ACT_PER_ELEM
                                      + _ACT_PER_OP)
                            # mask the 128-wide diagonal transition blocks
                            pe_ns = 0.0
                            for j, (kt, so) in enumerate(((ka, soa),
                                                          (kb, sob))):
                                if kt >= 4 * c:
                                    nc.vector.tensor_mul(
                                        ex[:, j, so:so + P],
                                        ex[:, j, so:so + P],
                                        tri_sb[:])
                                pe_ns += (QC - so) * _PE_PER_ROW
                            # act-debt bookkeeping + filler dispatch
                            debt[0] += act_ns - pe_ns
                            while fillers and debt[0] > fillers[0][1]:
                                t, tns = fillers.popleft()
                                t()
                                debt[0] -= tns
                    # PV + normalize; one sequential PSUM accumulation
                    # group per q-subtile (a PSUM bank allows only one
                    # open accumulation group at a time)
                    for h in (0, 1):
                        lh = 2 * p + h
                        y_ps = y_pool.tile([P, 4, P], f32, name="yps",
                                           tag="yps")
                        for qs in range(4):
                            last_kt = 4 * c + qs
                            for kt, so, ex, j in pieces[h]:
                                if kt > last_kt:
                                    continue
                                jj = kt - 4 * c
                                if jj >= 0 and qs < jj:
                                    continue
                                nc.tensor.matmul(
                                    y_ps[:, qs, 0:65],
                                    ex[:, j, ts(qs, P)],
                                    v_sb[kt][:, lh, :],
                                    start=(kt == 0),
                                    stop=(kt == last_kt))
                                debt[0] -= 65 * _PE_PER_ROW
                            while fillers and debt[0] > fillers[0][1]:
                                t, tns = fillers.popleft()
                                t()
                                debt[0] -= tns
                        # normalize: recip of denominators, scale 64 v-dims
                        r = r_pool.tile([P, 4], f32, name="rcp", tag="rcp")
                        nc.vector.reciprocal(out=r[:],
                                             in_=y_ps[:, :, 64:65])
                        for qs in range(4):
                            nc.vector.tensor_scalar_mul(
                                out=ysub[qs][:, h, :],
                                in0=y_ps[:, qs, 0:64],
                                scalar1=r[:, qs:qs + 1])
                    # transpose y [128 q, 128 dims] -> yT [128 dims, 128 q]
                    for qs in range(4):
                        nc.sync.dma_start(out=yT[p][c][:, ts(qs, P)],
                                          in_=ysub[qs][:], transpose=True)

            def proj_thunks(c, split_dma=False):
                """Partial out-proj for T-tiles of chunk c (all pairs)."""
                thunks = []
                osb = {}

                def mk(tt, half):
                    def t():
                        if half == 0 and not split_dma:
                            osb[tt] = o_pool.tile([P, 1024], f16, name="osb",
                                                  tag="osb")
                        ps = lin_pool.tile([P, QC], f32, name="lps",
                                           tag="lps")
                        for p in range(PAIRS):
                            nc.tensor.matmul(
                                ps[:], yT[p][tt // 4][:, ts(tt % 4, P)],
                                wp_sb[p][:, ts(half, QC)],
                                start=(p == 0), stop=(p == PAIRS - 1))
                        if split_dma:
                            ob = o_pool.tile([P, 1024], f16, name="osb",
                                             tag="osb")
                            nc.vector.tensor_copy(out=ob[:, 0:QC], in_=ps[:])
                            nc.sync.dma_start(
                                out=out[ts(tt, P), ts(half, QC)],
                                in_=ob[:, 0:QC])
                        else:
                            nc.vector.tensor_copy(
                                out=osb[tt][:, ts(half, QC)], in_=ps[:])
                            if half == 1:
                                nc.sync.dma_start(out=out[ts(tt, P), :],
                                                  in_=osb[tt][:])
                    return t

                for tt in range(4 * c, 4 * c + 4):
                    for half in (0, 1):
                        thunks.append((mk(tt, half),
                                       PAIRS * QC * _PE_PER_ROW))
                return thunks

            # ---------------- main schedule ----------------
            debt = [0.0]
            load_x_chunk(1)
            fillers = collections.deque(qkv_chunk_thunks(1))
            attn_chunk(0, range(PAIRS), fillers, debt)
            while fillers:
                fillers.popleft()[0]()

            load_x_chunk(2)
            fillers = collections.deque(qkv_chunk_thunks(2))
            attn_chunk(1, range(PAIRS), fillers, debt)
            while fillers:
                fillers.popleft()[0]()

            load_x_chunk(3)
            fillers = collections.deque(qkv_chunk_thunks(3))
            attn_chunk(2, range(PAIRS), fillers, debt)
            while fillers:
                fillers.popleft()[0]()

            fillers = collections.deque(
                proj_thunks(0) + proj_thunks(1) + proj_thunks(2))
            attn_chunk(3, range(PAIRS), fillers, debt)
            while fillers:
                fillers.popleft()[0]()

            for t, _ in proj_thunks(3):
                t()

    nc.compile()
    return nc


def _make_runner(nc):
    """Reusable 8-core SPMD runner (jit built once)."""
    import jax
    from jax.sharding import Mesh, PartitionSpec
    from jax.experimental.shard_map import shard_map
    from concourse import bass2jax
    import concourse.mybir as mybir

    bass2jax.install_neuronx_cc_hook()
    partition_name = (nc.partition_id_tensor.name
                      if nc.partition_id_tensor else None)
    in_names, out_names, out_avals, zero_outs = [], [], [], []
    for alloc in nc.m.functions[0].allocations:
        if not isinstance(alloc, mybir.MemoryLocationSet):
            continue
        name = alloc.memorylocations[0].name
        if alloc.kind == "ExternalInput":
            if name != partition_name:
                in_names.append(name)
        elif alloc.kind == "ExternalOutput":
            shape = tuple(alloc.tensor_shape)
            dtype = mybir.dt.np(alloc.dtype)
            out_names.append(name)
            out_avals.append(jax.core.ShapedArray(shape, dtype))
            zero_outs.append(np.zeros(shape, dtype))
    n_params = len(in_names)
    n_outs = len(out_avals)
    all_in = list(in_names) + list(out_names)
    if partition_name is not None:
        all_in.append(partition_name)

    def _body(*args):
        operands = list(args)
        if partition_name is not None:
            operands.append(bass2jax.partition_id_tensor())
        outs = bass2jax._bass_exec_p.bind(
            *operands,
            out_avals=tuple(out_avals),
            in_names=tuple(all_in),
            out_names=tuple(out_names),
            lowering_input_output_aliases=(),
            sim_require_finite=True,
            sim_require_nnan=True,
            nc=nc,
        )
        return tuple(outs)

    devices = jax.devices()[:N_CORES]
    mesh = Mesh(np.asarray(devices), ("core",))
    in_specs = (PartitionSpec("core"),) * (n_params + n_outs)
    out_specs = (PartitionSpec("core"),) * n_outs
    donate = tuple(range(n_params, n_params + n_outs))
    sharded = jax.jit(
        shard_map(_body, mesh=mesh, in_specs=in_specs, out_specs=out_specs,
                  check_rep=False),
        donate_argnums=donate, keep_unused=True)

    def run(in_maps):
        per_core = [[np.asarray(m[k]) for k in in_names] for m in in_maps]
        concat_in = [
            np.concatenate([per_core[c][i] for c in range(N_CORES)], axis=0)
            for i in range(n_params)]
        concat_zeros = [
            np.zeros((N_CORES * z.shape[0], *z.shape[1:]), z.dtype)
            for z in zero_outs]
        outs = sharded(*concat_in, *concat_zeros)
        jax.block_until_ready(outs)
        return [
            {name: np.asarray(outs[i]).reshape(N_CORES, *out_avals[i].shape)[c]
             for i, name in enumerate(out_names)}
            for c in range(N_CORES)]

    return run


def kernel(x, w_qkv, b_qkv, w_proj, b_proj):
    x = np.asarray(x, dtype=np.float32)
    w_qkv = np.asarray(w_qkv, dtype=np.float32)
    b_qkv = np.asarray(b_qkv, dtype=np.float32)
    w_proj = np.asarray(w_proj, dtype=np.float32)
    b_proj = np.asarray(b_proj, dtype=np.float32)

    w_q, w_k, w_v = w_qkv[0:1024], w_qkv[1024:2048], w_qkv[2048:3072]
    b_q, b_k, b_v = b_qkv[0:1024], b_qkv[1024:2048], b_qkv[2048:3072]
    has_qk_bias = bool(np.any(b_q) or np.any(b_k))

    key = ("runner", has_qk_bias)
    if key not in _RUNNER_CACHE:
        nc = _build(has_qk_bias)
        _RUNNER_CACHE[key] = _make_runner(nc)
    run = _RUNNER_CACHE[key]

    # causal transition-block mask: tri[k, i] = 1.0 iff k <= i
    kk = np.arange(P)
    tri = (kk[:, None] <= kk[None, :]).astype(np.float16)

    in_maps = []
    for core in range(N_CORES):
        b, g = divmod(core, 2)
        xT_c = x[b].T.astype(np.float16)  # [1024, 2048]
        if has_qk_bias:
            xT_c = np.concatenate(
                [xT_c, np.ones((1, T), np.float16)], axis=0)
        KD = D_MODEL + (1 if has_qk_bias else 0)
        wa_c = np.empty((KD, W_COLS), np.float32)
        wp_c = np.empty((512, 1024), np.float32)
        for p in range(PAIRS):
            hA = 8 * g + 2 * p
            hB = hA + 1
            cols = p * 256
            wa_c[:D_MODEL, cols + 0:cols + 64] = w_q[hA * 64:(hA + 1) * 64].T
            wa_c[:D_MODEL, cols + 64:cols + 128] = w_q[hB * 64:(hB + 1) * 64].T
            wa_c[:D_MODEL, cols + 128:cols + 192] = w_k[hA * 64:(hA + 1) * 64].T
            wa_c[:D_MODEL, cols + 192:cols + 256] = w_k[hB * 64:(hB + 1) * 64].T
            if has_qk_bias:
                wa_c[D_MODEL, cols + 0:cols + 64] = b_q[hA * 64:(hA + 1) * 64]
                wa_c[D_MODEL, cols + 64:cols + 128] = b_q[hB * 64:(hB + 1) * 64]
                wa_c[D_MODEL, cols + 128:cols + 192] = b_k[hA * 64:(hA + 1) * 64]
                wa_c[D_MODEL, cols + 192:cols + 256] = b_k[hB * 64:(hB + 1) * 64]
            # wp rows pair-major: [hA dims 0..63 | hB dims 64..127]
            wp_c[p * 128:p * 128 + 64, :] = w_proj.T[hA * 64:(hA + 1) * 64, :]
            wp_c[p * 128 + 64:p * 128 + 128, :] = \
                w_proj.T[hB * 64:(hB + 1) * 64, :]
        # v columns, head-major for the group
        for lh in range(8):
            head = 8 * g + lh
            wa_c[:D_MODEL, 1024 + lh * 64:1024 + (lh + 1) * 64] = \
                w_v[head * 64:(head + 1) * 64].T
            if has_qk_bias:
                wa_c[D_MODEL, 1024 + lh * 64:1024 + (lh + 1) * 64] = \
                    b_v[head * 64:(head + 1) * 64]
        in_maps.append({
            "xT": xT_c,
            "wa": wa_c.astype(np.float16),
            "wp": wp_c.astype(np.float16),
            "tri": tri,
        })

    results = run(in_maps)

    # partial-sum unshard: the two head-group cores of each batch each
    # produced out_partial[T, 1024]; add them.
    out = np.empty((B, T, D_MODEL), dtype=np.float32)
    for b in range(B):
        out[b] = (results[2 * b]["out"].astype(np.float32)
                  + results[2 * b + 1]["out"].astype(np.float32))

    # exact host-side bias folds (v-bias rides softmax row-sums == 1 and is
    # on-device in the qk-bias build; proj bias is additive)
    if np.any(b_v) and not has_qk_bias:
        out += (b_v @ w_proj.T)[None, None, :]
    if np.any(b_proj):
        out += b_proj[None, None, :]
    return out


# revision 4
# speedup vs baseline: 1.0374x; 1.0206x over previous
"""Causal self-attention (B=4, T=2048, C=1024, H=16) on 8 TRN2 NeuronCores.

Sharding: core = 2*b + g (b = batch 0..3, g = head-group 0..1). Each core
computes qkv + attention for its batch and its 8 heads, then a PARTIAL
output projection over its own 512 y-dims for ALL 1024 output columns.
The host adds the two partials per batch (partial-sum unsharding) - no
device collectives at all.

All matmuls run in fp16 (1 PE cycle/row, no min-width constraint),
accumulation in fp32 PSUM. Attention PV uses the cheap orientation
out[q,65] = ex^T @ [V | 1] (65 rows per (k-tile, q-subtile) instead of
512), with the softmax denominator from the ones column; y is normalized
with a per-partition scalar multiply and transposed back to [dims, T]
with the DMA XBAR transpose (off the PE).

The attention inner loop is Activation-engine paced (exp); qkv-proj and
out-proj matmul "filler" work is interleaved between score/PV groups via
a debt counter so the PE never idles waiting for exp.
"""
import collections
import numpy as np

D_MODEL = 1024
N_HEAD = 16
D_HEAD = 64
B = 4
T = 2048
N_CORES = 8
P = 128
PAIRS = 4          # head pairs per core
NQ = 4             # q-chunks of 512
QC = 512           # q chunk width
KT = D_MODEL // P  # 8 contraction tiles for the qkv projection
W_COLS = 1536      # 1024 q/k cols + 512 v cols per core

_RUNNER_CACHE = {}

# cost-model-ish estimates (ns) for the act-debt interleaver
_ACT_PER_ELEM = 0.833
_ACT_PER_OP = 185.0
_PE_PER_ROW = 0.4167


def _build(has_qk_bias: bool, _nphases: int = 5):
    from concourse import bacc
    import concourse.mybir as mybir
    from concourse.tile import TileContext
    from concourse.bass import ts

    f32 = mybir.dt.float32
    f16 = mybir.dt.float16
    KD = D_MODEL + (1 if has_qk_bias else 0)

    nc = bacc.Bacc("TRN2", target_bir_lowering=False, debug=False,
                   num_devices=N_CORES)
    xT = nc.dram_tensor("xT", [KD, T], f16, kind="ExternalInput")
    wa = nc.dram_tensor("wa", [KD, W_COLS], f16, kind="ExternalInput")
    wp = nc.dram_tensor("wp", [512, 1024], f16, kind="ExternalInput")
    tri = nc.dram_tensor("tri", [P, P], f16, kind="ExternalInput")
    idn = nc.dram_tensor("idn", [P, P], f32, kind="ExternalInput")
    out = nc.dram_tensor("out", [T, 1024], f16, kind="ExternalOutput")

    EXPF = mybir.ActivationFunctionType.Exp

    with TileContext(nc) as tc:
        with (
            tc.tile_pool(name="wts", bufs=1) as wts,
            tc.tile_pool(name="qk_res", bufs=1) as qk_res,
            tc.tile_pool(name="v_res", bufs=1) as v_res,
            tc.tile_pool(name="yt_res", bufs=1) as yt_res,
            tc.tile_pool(name="xs", bufs=2) as xs_pool,
            tc.tile_pool(name="exp", bufs=20) as ex_pool,
            tc.tile_pool(name="ysb", bufs=8) as ysb_pool,
            tc.tile_pool(name="rcp", bufs=4) as r_pool,
            tc.tile_pool(name="osb", bufs=4) as o_pool,
            tc.tile_pool(name="lin", bufs=2, space="PSUM") as lin_pool,
            tc.tile_pool(name="st", bufs=2, space="PSUM") as st_pool,
            tc.tile_pool(name="yps", bufs=2, space="PSUM") as y_pool,
        ):
            # ---------------- persistent tiles ----------------
            # wa_sb[i] holds contraction k-tiles 2i, 2i+1: [128, 2, 1536]
            wa_sb = [wts.tile([P, 2, W_COLS], f16, name=f"wa{i}")
                     for i in range(4)]
            wp_sb = [wts.tile([P, 1024], f16, name=f"wp{j}")
                     for j in range(PAIRS)]
            tri_sb = wts.tile([P, P], f16, name="tri_sb")
            idn_sb = wts.tile([P, P], f32, name="idn_sb")
            if has_qk_bias:
                xrow = wts.tile([1, T], f16, name="xrow")
                wrow = wts.tile([1, W_COLS], f16, name="wrow")
            # qT[p][c]/kT[p][c]: [128 dims (2 heads x 64), 512 T]
            qT = [[qk_res.tile([P, QC], f16, name=f"qT{p}_{c}")
                   for c in range(NQ)] for p in range(PAIRS)]
            kT = [[qk_res.tile([P, QC], f16, name=f"kT{p}_{c}")
                   for c in range(NQ)] for p in range(PAIRS)]
            # v_sb[tt]: [128 keys, 8 heads, 65 (v | 1)]
            v_sb = [v_res.tile([P, 8, 65], f16, name=f"v{t}")
                    for t in range(T // P)]
            # yT[p][c]: [128 dims, 512 T] (normalized, transposed)
            yT = [[yt_res.tile([P, QC], f16, name=f"yT{p}_{c}")
                   for c in range(NQ)] for p in range(PAIRS)]

            # ---------------- DMA loads ----------------
            # x chunk tiles; chunk 0 split in 4 pieces for early start
            xt = [None] * NQ
            xt[0] = xs_pool.tile([P, KT, QC], f16, name="xt", tag="xt")

            def load_x_piece(i):
                nc.sync.dma_start(
                    out=xt[0][:, 2 * i:2 * i + 2, :],
                    in_=xT[ts(i, 2 * P), ts(0, QC)].rearrange(
                        "(j p) q -> p j q", p=P))

            def load_wa(i, j):
                nc.sync.dma_start(
                    out=wa_sb[i][:, j, :],
                    in_=wa[ts(2 * i + j, P), :])

            # chunk-0 weight loads arrive in column-waves matching the
            # k-outer passes: wave 1 = qk cols 0:512 (pass A), wave 2 =
            # cols 512:1024 (pass B), wave 3 = v cols (pass C)
            def load_wa_cols(k, cc):
                nc.sync.dma_start(
                    out=wa_sb[k // 2][:, k % 2, ts(cc, QC)],
                    in_=wa[ts(k, P), ts(cc, QC)])

            load_wa_cols(0, 0)
            for i in range(4):
                load_x_piece(i)
                if 2 * i + 1 < KT:
                    load_wa_cols(2 * i + 1, 0)
                if 2 * i + 2 < KT:
                    load_wa_cols(2 * i + 2, 0)
            def load_x_chunk(n):
                xt[n] = xs_pool.tile([P, KT, QC], f16, name="xt", tag="xt")
                nc.sync.dma_start(
                    out=xt[n],
                    in_=xT[0:D_MODEL, ts(n, QC)].rearrange(
                        "(k p) q -> p k q", p=P))

            for k in range(KT):
                load_wa_cols(k, 1)
            for k in range(KT):
                load_wa_cols(k, 2)
            nc.sync.dma_start(out=tri_sb, in_=tri[:])
            nc.sync.dma_start(out=idn_sb, in_=idn[:])
            load_x_chunk(1)
            for j in range(PAIRS):
                nc.sync.dma_start(out=wp_sb[j], in_=wp[ts(j, P), :])
            if has_qk_bias:
                nc.sync.dma_start(out=xrow, in_=xT[D_MODEL:D_MODEL + 1, :])
                nc.sync.dma_start(out=wrow, in_=wa[D_MODEL:D_MODEL + 1, :])

            # ---------------- qkv projection helpers ----------------
            # w block col ranges: block 2p -> q pair p, 2p+1 -> k pair p,
            # block 8+j -> v (cols 1024 + j*128)
            def qk_copy_dest(blk, n):
                p, kind = divmod(blk, 2)
                return (qT if kind == 0 else kT)[p][n]

            def emit_qk_block(n, blk, ps):
                """Accumulate w-block x x-chunk into ps and copy to SBUF."""
                for k in range(KT):
                    nc.tensor.matmul(
                        ps[:], wa_sb[k // 2][:, k % 2, ts(blk, P)],
                        xt[n][:, k, :],
                        start=(k == 0),
                        stop=(k == KT - 1) and not has_qk_bias)
                if has_qk_bias:
                    nc.tensor.matmul(
                        ps[:], wrow[:, ts(blk, P)], xrow[:, ts(n, QC)],
                        start=False, stop=True)
                nc.vector.tensor_copy(out=qk_copy_dest(blk, n), in_=ps[:])

            def emit_v_block(n, tsub, ps):
                tt = 4 * n + tsub
                for k in range(KT):
                    nc.tensor.matmul(
                        ps[:], xt[n][:, k, ts(tsub, P)],
                        wa_sb[k // 2][:, k % 2, 1024:1536],
                        start=(k == 0),
                        stop=(k == KT - 1) and not has_qk_bias)
                if has_qk_bias:
                    nc.tensor.matmul(
                        ps[:], xrow[:, n * QC + tsub * P:
                                    n * QC + (tsub + 1) * P],
                        wrow[:, 1024:1536], start=False, stop=True)
                nc.vector.memset(v_sb[tt][:, :, 64:65], 1.0)
                nc.vector.tensor_copy(
                    out=v_sb[tt][:, :, 0:64],
                    in_=ps.rearrange("p (h c) -> p h c", c=64))

            def qkv_chunk_thunks(n):
                """Filler thunks for chunk n (needs xt[n] loaded)."""
                thunks = []
                for blk in range(8):
                    def t(blk=blk):
                        ps = lin_pool.tile([P, QC], f32, name="lps",
                                           tag="lps")
                        emit_qk_block(n, blk, ps)
                    thunks.append((t, 8 * QC * _PE_PER_ROW))
                for tsub in range(4):
                    def t(tsub=tsub):
                        ps = lin_pool.tile([P, QC], f32, name="lps",
                                           tag="lps")
                        emit_v_block(n, tsub, ps)
                    thunks.append((t, 8 * QC * _PE_PER_ROW))
                return thunks

            # ---------------- chunk 0: k-outer for early start ----------
            # 3 passes of 4 accumulators (2 lin tiles + 2 halves of an st
            # tile) so compute streams while wa/x DMAs land.
            for pass_blocks in ([0, 1, 2, 3], [4, 5, 6, 7],
                                [(0, 0), (0, 1), (0, 2), (0, 3)]):
                l0 = lin_pool.tile([P, QC], f32, name="lps", tag="lps")
                l1 = lin_pool.tile([P, QC], f32, name="lps", tag="lps")
                s0 = st_pool.tile([P, 2, QC], f32, name="st", tag="st")
                accs = [l0, l1, s0[:, 0, :], s0[:, 1, :]]
                is_v = isinstance(pass_blocks[0], tuple)
                for k in range(KT):
                    for a, blk in zip(accs, pass_blocks):
                        if is_v:
                            _, tsub = blk
                            nc.tensor.matmul(
                                a[:], xt[0][:, k, ts(tsub, P)],
                                wa_sb[k // 2][:, k % 2, 1024:1536],
                                start=(k == 0),
                                stop=(k == KT - 1) and not has_qk_bias)
                        else:
                            nc.tensor.matmul(
                                a[:], wa_sb[k // 2][:, k % 2, ts(blk, P)],
                                xt[0][:, k, :],
                                start=(k == 0),
                                stop=(k == KT - 1) and not has_qk_bias)
                for a, blk in zip(accs, pass_blocks):
                    if is_v:
                        _, tsub = blk
                        if has_qk_bias:
                            nc.tensor.matmul(
                                a[:], xrow[:, ts(tsub, P)],
                                wrow[:, 1024:1536], start=False, stop=True)
                        nc.vector.memset(v_sb[tsub][:, :, 64:65], 1.0)
                        nc.vector.tensor_copy(
                            out=v_sb[tsub][:, :, 0:64],
                            in_=a.rearrange("p (h c) -> p h c", c=64))
                    else:
                        if has_qk_bias:
                            nc.tensor.matmul(
                                a[:], wrow[:, ts(blk, P)], xrow[:, ts(0, QC)],
                                start=False, stop=True)
                        nc.vector.tensor_copy(out=qk_copy_dest(blk, 0),
                                              in_=a[:])

            # ---------------- attention ----------------
            def attn_chunk(c, pair_list, fillers, debt):
                """Attention for q-chunk c over pairs in pair_list.

                fillers: deque of (thunk, pe_ns); popped when the act-debt
                (exp time not covered by attention PE work) exceeds one
                thunk's worth, keeping the PE busy while Act catches up.
                """
                # groups: (kt_a, kt_b, so_a, so_b)
                groups = [(2 * g, 2 * g + 1, 0, 0) for g in range(2 * c)]
                groups.append((4 * c, 4 * c + 1, 0, P))
                groups.append((4 * c + 2, 4 * c + 3, 2 * P, 3 * P))
                for p in pair_list:
                    if c == 3 and p == pair_list[-1]:
                        ysubf = [ysb_pool.tile([P, 2, 64], f32, name="ysbf",
                                               tag="ysbf") for _ in range(4)]
                    else:
                        ysub = [ysb_pool.tile([P, 2, 64], f16, name="ysb",
                                              tag="ysb") for _ in range(4)]
                    pieces = {0: [], 1: []}  # h -> (kt, so, ex_tile, j)
                    for h in (0, 1):
                        hb = h * 64
                        for ka, kb, soa, sob in groups:
                            st = st_pool.tile([P, 2, QC], f32, name="st",
                                              tag="st")
                            ex = ex_pool.tile([P, 2, QC], f16, name="ex",
                                              tag="ex")
                            for j, (kt, so) in enumerate(((ka, soa),
                                                          (kb, sob))):
                                nc.tensor.matmul(
                                    st[:, j, so:QC],
                                    kT[p][kt // 4][hb:hb + 64,
                                                   ts(kt % 4, P)],
                                    qT[p][c][hb:hb + 64, so:QC],
                                    start=True, stop=True)
                                pieces[h].append((kt, so, ex, j))
                            # exp: one op per group over [min_so:512] of
                            # both pieces; the [soa:sob) part of piece b is
                            # unwritten PSUM whose exp is never read by PV
                            mso = min(soa, sob)
                            nc.scalar.activation(ex[:, :, mso:QC],
                                                 st[:, :, mso:QC], EXPF,
                                                 scale=0.125)
                            act_ns = (2 * (QC - mso) * _ACT_PER_ELEM
                                      + _ACT_PER_OP)
                            # mask the 128-wide diagonal transition blocks
                            pe_ns = 0.0
                            for j, (kt, so) in enumerate(((ka, soa),
                                                          (kb, sob))):
                                if kt >= 4 * c:
                                    nc.vector.tensor_mul(
                                        ex[:, j, so:so + P],
                                        ex[:, j, so:so + P],
                                        tri_sb[:])
                                pe_ns += (QC - so) * _PE_PER_ROW
                            # act-debt bookkeeping + filler dispatch
                            debt[0] += act_ns - pe_ns
                            while fillers and debt[0] > fillers[0][1]:
                                t, tns = fillers.popleft()
                                t()
                                debt[0] -= tns
                    # PV + normalize; one sequential PSUM accumulation
                    # group per q-subtile (a PSUM bank allows only one
                    # open accumulation group at a time)
                    for h in (0, 1):
                        lh = 2 * p + h
                        y_ps = y_pool.tile([P, 4, P], f32, name="yps",
                                           tag="yps")
                        for qs in range(4):
                            last_kt = 4 * c + qs
                            for kt, so, ex, j in pieces[h]:
                                if kt > last_kt:
                                    continue
                                jj = kt - 4 * c
                                if jj >= 0 and qs < jj:
                                    continue
                                nc.tensor.matmul(
                                    y_ps[:, qs, 0:65],
                                    ex[:, j, ts(qs, P)],
                                    v_sb[kt][:, lh, :],
                                    start=(kt == 0),
                                    stop=(kt == last_kt))
                                debt[0] -= 65 * _PE_PER_ROW
                            while fillers and debt[0] > fillers[0][1]:
                                t, tns = fillers.popleft()
                                t()
                                debt[0] -= tns
                        # normalize: recip of denominators, scale 64 v-dims
                        r = r_pool.tile([P, 4], f32, name="rcp", tag="rcp")
                        nc.vector.reciprocal(out=r[:],
                                             in_=y_ps[:, :, 64:65])
                        # final pair of the last chunk: normalize on DVE
                        # into f32 staging (feeds the f32 PE transpose on
                        # the tail path)
                        dst = (ysubf if (c == 3 and p == pair_list[-1])
                               else ysub)
                        for qs in range(4):
                            nc.vector.tensor_scalar_mul(
                                out=dst[qs][:, h, :],
                                in0=y_ps[:, qs, 0:64],
                                scalar1=r[:, qs:qs + 1])
                    # transpose y [128 q, 128 dims] -> yT [128 dims, 128 q]
                    if c == 3 and p == pair_list[-1]:
                        # tail-latency path: f32 PE transpose through a free
                        # st PSUM slice + DVE copy (avoids 4 serialized
                        # HWDGE ops right before the proj(3) closes)
                        tp = st_pool.tile([P, 2, QC], f32, name="st",
                                          tag="st")
                        for qs in range(4):
                            nc.tensor.transpose(
                                tp[:, qs % 2, 0:P], ysubf[qs][:],
                                idn_sb[:])
                            nc.vector.tensor_copy(
                                out=yT[p][c][:, ts(qs, P)],
                                in_=tp[:, qs % 2, 0:P])
                    else:
                        for qs in range(4):
                            nc.sync.dma_start(out=yT[p][c][:, ts(qs, P)],
                                              in_=ysub[qs][:],
                                              transpose=True)

            def proj_thunks(c, split_dma=False):
                """Partial out-proj for T-tiles of chunk c (all pairs)."""
                thunks = []
                osb = {}

                def mk(tt, half):
                    def t():
                        if half == 0 and not split_dma:
                            osb[tt] = o_pool.tile([P, 1024], f16, name="osb",
                                                  tag="osb")
                        ps = lin_pool.tile([P, QC], f32, name="lps",
                                           tag="lps")
                        for p in range(PAIRS):
                            nc.tensor.matmul(
                                ps[:], yT[p][tt // 4][:, ts(tt % 4, P)],
                                wp_sb[p][:, ts(half, QC)],
                                start=(p == 0), stop=(p == PAIRS - 1))
                        if split_dma:
                            ob = o_pool.tile([P, 1024], f16, name="osb",
                                             tag="osb")
                            nc.vector.tensor_copy(out=ob[:, 0:QC], in_=ps[:])
                            nc.sync.dma_start(
                                out=out[ts(tt, P), ts(half, QC)],
                                in_=ob[:, 0:QC])
                        else:
                            nc.vector.tensor_copy(
                                out=osb[tt][:, ts(half, QC)], in_=ps[:])
                            if half == 1:
                                nc.sync.dma_start(out=out[ts(tt, P), :],
                                                  in_=osb[tt][:])
                    return t

                for tt in range(4 * c, 4 * c + 4):
                    for half in (0, 1):
                        thunks.append((mk(tt, half),
                                       PAIRS * QC * _PE_PER_ROW))
                return thunks

            # ---------------- main schedule ----------------
            debt = [0.0]
            fillers = collections.deque(qkv_chunk_thunks(1))
            attn_chunk(0, range(PAIRS), fillers, debt)
            while fillers:
                fillers.popleft()[0]()

            load_x_chunk(2)
            fillers = collections.deque(qkv_chunk_thunks(2))
            attn_chunk(1, range(PAIRS), fillers, debt)
            while fillers:
                fillers.popleft()[0]()

            load_x_chunk(3)
            fillers = collections.deque(qkv_chunk_thunks(3))
            attn_chunk(2, range(PAIRS), fillers, debt)
            while fillers:
                fillers.popleft()[0]()

            fillers = collections.deque(
                proj_thunks(0) + proj_thunks(1) + proj_thunks(2))
            attn_chunk(3, range(PAIRS), fillers, debt)
            while fillers:
                fillers.popleft()[0]()

            # proj(3): keep two groups' pair-0..2 matmuls in flight ahead
            # of the pair-3 close so the last transposes are hidden
            open_g = collections.deque()

            def open_group(tt, half):
                ps = lin_pool.tile([P, QC], f32, name="lps", tag="lps")
                for p in range(PAIRS - 1):
                    nc.tensor.matmul(
                        ps[:], yT[p][tt // 4][:, ts(tt % 4, P)],
                        wp_sb[p][:, ts(half, QC)],
                        start=(p == 0), stop=False)
                open_g.append((tt, half, ps))

            osb3 = {}

            def close_group():
                tt, half, ps = open_g.popleft()
                nc.tensor.matmul(
                    ps[:], yT[PAIRS - 1][tt // 4][:, ts(tt % 4, P)],
                    wp_sb[PAIRS - 1][:, ts(half, QC)],
                    start=False, stop=True)
                if half == 0:
                    osb3[tt] = o_pool.tile([P, 1024], f16, name="osb",
                                           tag="osb")
                nc.vector.tensor_copy(out=osb3[tt][:, ts(half, QC)],
                                      in_=ps[:])
                if half == 1:
                    nc.sync.dma_start(out=out[ts(tt, P), :],
                                      in_=osb3[tt][:])

            for tt in range(12, 16):
                for half in (0, 1):
                    open_group(tt, half)
                    if len(open_g) == 2:
                        close_group()
            while open_g:
                close_group()

    nc.compile()
    return nc


def _make_runner(nc):
    """Reusable 8-core SPMD runner (jit built once)."""
    import jax
    from jax.sharding import Mesh, PartitionSpec
    from jax.experimental.shard_map import shard_map
    from concourse import bass2jax
    import concourse.mybir as mybir

    bass2jax.install_neuronx_cc_hook()
    partition_name = (nc.partition_id_tensor.name
                      if nc.partition_id_tensor else None)
    in_names, out_names, out_avals, zero_outs = [], [], [], []
    for alloc in nc.m.functions[0].allocations:
        if not isinstance(alloc, mybir.MemoryLocationSet):
            continue
        name = alloc.memorylocations[0].name
        if alloc.kind == "ExternalInput":
            if name != partition_name:
                in_names.append(name)
        elif alloc.kind == "ExternalOutput":
            shape = tuple(alloc.tensor_shape)
            dtype = mybir.dt.np(alloc.dtype)
            out_names.append(name)
            out_avals.append(jax.core.ShapedArray(shape, dtype))
            zero_outs.append(np.zeros(shape, dtype))
    n_params = len(in_names)
    n_outs = len(out_avals)
    all_in = list(in_names) + list(out_names)
    if partition_name is not None:
        all_in.append(partition_name)

    def _body(*args):
        operands = list(args)
        if partition_name is not None:
            operands.append(bass2jax.partition_id_tensor())
        outs = bass2jax._bass_exec_p.bind(
            *operands,
            out_avals=tuple(out_avals),
            in_names=tuple(all_in),
            out_names=tuple(out_names),
            lowering_input_output_aliases=(),
            sim_require_finite=True,
            sim_require_nnan=True,
            nc=nc,
        )
        return tuple(outs)

    devices = jax.devices()[:N_CORES]
    mesh = Mesh(np.asarray(devices), ("core",))
    in_specs = (PartitionSpec("core"),) * (n_params + n_outs)
    out_specs = (PartitionSpec("core"),) * n_outs
    donate = tuple(range(n_params, n_params + n_outs))
    sharded = jax.jit(
        shard_map(_body, mesh=mesh, in_specs=in_specs, out_specs=out_specs,
                  check_rep=False),
        donate_argnums=donate, keep_unused=True)

    def run(in_maps):
        per_core = [[np.asarray(m[k]) for k in in_names] for m in in_maps]
        concat_in = [
            np.concatenate([per_core[c][i] for c in range(N_CORES)], axis=0)
            for i in range(n_params)]
        concat_zeros = [
            np.zeros((N_CORES * z.shape[0], *z.shape[1:]), z.dtype)
            for z in zero_outs]
        outs = sharded(*concat_in, *concat_zeros)
        jax.block_until_ready(outs)
        return [
            {name: np.asarray(outs[i]).reshape(N_CORES, *out_avals[i].shape)[c]
             for i, name in enumerate(out_names)}
            for c in range(N_CORES)]

    return run


def kernel(x, w_qkv, b_qkv, w_proj, b_proj):
    x = np.asarray(x, dtype=np.float32)
    w_qkv = np.asarray(w_qkv, dtype=np.float32)
    b_qkv = np.asarray(b_qkv, dtype=np.float32)
    w_proj = np.asarray(w_proj, dtype=np.float32)
    b_proj = np.asarray(b_proj, dtype=np.float32)

    w_q, w_k, w_v = w_qkv[0:1024], w_qkv[1024:2048], w_qkv[2048:3072]
    b_q, b_k, b_v = b_qkv[0:1024], b_qkv[1024:2048], b_qkv[2048:3072]
    has_qk_bias = bool(np.any(b_q) or np.any(b_k))

    key = ("runner", has_qk_bias)
    if key not in _RUNNER_CACHE:
        nc = _build(has_qk_bias)
        _RUNNER_CACHE[key] = _make_runner(nc)
    run = _RUNNER_CACHE[key]

    # causal transition-block mask: tri[k, i] = 1.0 iff k <= i
    kk = np.arange(P)
    tri = (kk[:, None] <= kk[None, :]).astype(np.float16)
    idn = np.eye(P, dtype=np.float32)

    in_maps = []
    for core in range(N_CORES):
        b, g = divmod(core, 2)
        xT_c = x[b].T.astype(np.float16)  # [1024, 2048]
        if has_qk_bias:
            xT_c = np.concatenate(
                [xT_c, np.ones((1, T), np.float16)], axis=0)
        KD = D_MODEL + (1 if has_qk_bias else 0)
        wa_c = np.empty((KD, W_COLS), np.float32)
        wp_c = np.empty((512, 1024), np.float32)
        for p in range(PAIRS):
            hA = 8 * g + 2 * p
            hB = hA + 1
            cols = p * 256
            wa_c[:D_MODEL, cols + 0:cols + 64] = w_q[hA * 64:(hA + 1) * 64].T
            wa_c[:D_MODEL, cols + 64:cols + 128] = w_q[hB * 64:(hB + 1) * 64].T
            wa_c[:D_MODEL, cols + 128:cols + 192] = w_k[hA * 64:(hA + 1) * 64].T
            wa_c[:D_MODEL, cols + 192:cols + 256] = w_k[hB * 64:(hB + 1) * 64].T
            if has_qk_bias:
                wa_c[D_MODEL, cols + 0:cols + 64] = b_q[hA * 64:(hA + 1) * 64]
                wa_c[D_MODEL, cols + 64:cols + 128] = b_q[hB * 64:(hB + 1) * 64]
                wa_c[D_MODEL, cols + 128:cols + 192] = b_k[hA * 64:(hA + 1) * 64]
                wa_c[D_MODEL, cols + 192:cols + 256] = b_k[hB * 64:(hB + 1) * 64]
            # wp rows pair-major: [hA dims 0..63 | hB dims 64..127]
            wp_c[p * 128:p * 128 + 64, :] = w_proj.T[hA * 64:(hA + 1) * 64, :]
            wp_c[p * 128 + 64:p * 128 + 128, :] = \
                w_proj.T[hB * 64:(hB + 1) * 64, :]
        # v columns, head-major for the group
        for lh in range(8):
            head = 8 * g + lh
            wa_c[:D_MODEL, 1024 + lh * 64:1024 + (lh + 1) * 64] = \
                w_v[head * 64:(head + 1) * 64].T
            if has_qk_bias:
                wa_c[D_MODEL, 1024 + lh * 64:1024 + (lh + 1) * 64] = \
                    b_v[head * 64:(head + 1) * 64]
        in_maps.append({
            "xT": xT_c,
            "wa": wa_c.astype(np.float16),
            "wp": wp_c.astype(np.float16),
            "tri": tri,
            "idn": idn,
        })

    results = run(in_maps)

    # partial-sum unshard: the two head-group cores of each batch each
    # produced out_partial[T, 1024]; add them.
    out = np.empty((B, T, D_MODEL), dtype=np.float32)
    for b in range(B):
        out[b] = (results[2 * b]["out"].astype(np.float32)
                  + results[2 * b + 1]["out"].astype(np.float32))

    # exact host-side bias folds (v-bias rides softmax row-sums == 1 and is
    # on-device in the qk-bias build; proj bias is additive)
    if np.any(b_v) and not has_qk_bias:
        out += (b_v @ w_proj.T)[None, None, :]
    if np.any(b_proj):
        out += b_proj[None, None, :]
    return out


# revision 5
# speedup vs baseline: 1.0398x; 1.0024x over previous
"""Causal self-attention (B=4, T=2048, C=1024, H=16) on 8 TRN2 NeuronCores.

Sharding: core = 2*b + g (b = batch 0..3, g = head-group 0..1). Each core
computes qkv + attention for its batch and its 8 heads, then a PARTIAL
output projection over its own 512 y-dims for ALL 1024 output columns.
The host adds the two partials per batch (partial-sum unsharding) - no
device collectives at all.

All matmuls run in fp16 (1 PE cycle/row, no min-width constraint),
accumulation in fp32 PSUM. Attention PV uses the cheap orientation
out[q,65] = ex^T @ [V | 1] (65 rows per (k-tile, q-subtile) instead of
512), with the softmax denominator from the ones column; y is normalized
with a per-partition scalar multiply and transposed back to [dims, T]
with the DMA XBAR transpose (off the PE).

The attention inner loop is Activation-engine paced (exp); qkv-proj and
out-proj matmul "filler" work is interleaved between score/PV groups via
a debt counter so the PE never idles waiting for exp.
"""
import collections
import numpy as np

D_MODEL = 1024
N_HEAD = 16
D_HEAD = 64
B = 4
T = 2048
N_CORES = 8
P = 128
PAIRS = 4          # head pairs per core
NQ = 4             # q-chunks of 512
QC = 512           # q chunk width
KT = D_MODEL // P  # 8 contraction tiles for the qkv projection
W_COLS = 1536      # 1024 q/k cols + 512 v cols per core

_RUNNER_CACHE = {}

# cost-model-ish estimates (ns) for the act-debt interleaver
_ACT_PER_ELEM = 0.833
_ACT_PER_OP = 185.0
_PE_PER_ROW = 0.4167


def _build(has_qk_bias: bool, _nphases: int = 5):
    from concourse import bacc
    import concourse.mybir as mybir
    from concourse.tile import TileContext
    from concourse.bass import ts

    f32 = mybir.dt.float32
    f16 = mybir.dt.float16
    KD = D_MODEL + (1 if has_qk_bias else 0)

    nc = bacc.Bacc("TRN2", target_bir_lowering=False, debug=False,
                   num_devices=N_CORES)
    xT = nc.dram_tensor("xT", [KD, T], f16, kind="ExternalInput")
    wa = nc.dram_tensor("wa", [KD, W_COLS], f16, kind="ExternalInput")
    wp = nc.dram_tensor("wp", [512, 1024], f16, kind="ExternalInput")
    tri = nc.dram_tensor("tri", [P, P], f16, kind="ExternalInput")
    idn = nc.dram_tensor("idn", [P, P], f32, kind="ExternalInput")
    out = nc.dram_tensor("out", [T, 1024], f16, kind="ExternalOutput")

    EXPF = mybir.ActivationFunctionType.Exp

    with TileContext(nc) as tc:
        with (
            tc.tile_pool(name="wts", bufs=1) as wts,
            tc.tile_pool(name="qk_res", bufs=1) as qk_res,
            tc.tile_pool(name="v_res", bufs=1) as v_res,
            tc.tile_pool(name="yt_res", bufs=1) as yt_res,
            tc.tile_pool(name="xs", bufs=2) as xs_pool,
            tc.tile_pool(name="exp", bufs=34) as ex_pool,
            tc.tile_pool(name="ysb", bufs=8) as ysb_pool,
            tc.tile_pool(name="rcp", bufs=4) as r_pool,
            tc.tile_pool(name="osb", bufs=4) as o_pool,
            tc.tile_pool(name="lin", bufs=2, space="PSUM") as lin_pool,
            tc.tile_pool(name="st", bufs=2, space="PSUM") as st_pool,
            tc.tile_pool(name="yps", bufs=2, space="PSUM") as y_pool,
        ):
            # ---------------- persistent tiles ----------------
            # wa_sb[i] holds contraction k-tiles 2i, 2i+1: [128, 2, 1536]
            wa_sb = [wts.tile([P, 2, W_COLS], f16, name=f"wa{i}")
                     for i in range(4)]
            wp_sb = [wts.tile([P, 1024], f16, name=f"wp{j}")
                     for j in range(PAIRS)]
            tri_sb = wts.tile([P, P], f16, name="tri_sb")
            idn_sb = wts.tile([P, P], f32, name="idn_sb")
            if has_qk_bias:
                xrow = wts.tile([1, T], f16, name="xrow")
                wrow = wts.tile([1, W_COLS], f16, name="wrow")
            # qT[p][c]/kT[p][c]: [128 dims (2 heads x 64), 512 T]
            qT = [[qk_res.tile([P, QC], f16, name=f"qT{p}_{c}")
                   for c in range(NQ)] for p in range(PAIRS)]
            kT = [[qk_res.tile([P, QC], f16, name=f"kT{p}_{c}")
                   for c in range(NQ)] for p in range(PAIRS)]
            # v_sb[tt]: [128 keys, 8 heads, 65 (v | 1)]
            v_sb = [v_res.tile([P, 8, 65], f16, name=f"v{t}")
                    for t in range(T // P)]
            # yT[p][c]: [128 dims, 512 T] (normalized, transposed)
            yT = [[yt_res.tile([P, QC], f16, name=f"yT{p}_{c}")
                   for c in range(NQ)] for p in range(PAIRS)]

            # ---------------- DMA loads ----------------
            # x chunk tiles; chunk 0 split in 4 pieces for early start
            xt = [None] * NQ
            xt[0] = xs_pool.tile([P, KT, QC], f16, name="xt", tag="xt")

            def load_x_piece(i):
                nc.sync.dma_start(
                    out=xt[0][:, 2 * i:2 * i + 2, :],
                    in_=xT[ts(i, 2 * P), ts(0, QC)].rearrange(
                        "(j p) q -> p j q", p=P))

            def load_wa(i, j):
                nc.sync.dma_start(
                    out=wa_sb[i][:, j, :],
                    in_=wa[ts(2 * i + j, P), :])

            # chunk-0 weight loads arrive in column-waves matching the
            # k-outer passes: wave 1 = qk cols 0:512 (pass A), wave 2 =
            # cols 512:1024 (pass B), wave 3 = v cols (pass C)
            def load_wa_cols(k, cc):
                nc.sync.dma_start(
                    out=wa_sb[k // 2][:, k % 2, ts(cc, QC)],
                    in_=wa[ts(k, P), ts(cc, QC)])

            load_wa_cols(0, 0)
            for i in range(4):
                load_x_piece(i)
                if 2 * i + 1 < KT:
                    load_wa_cols(2 * i + 1, 0)
                if 2 * i + 2 < KT:
                    load_wa_cols(2 * i + 2, 0)
            def load_x_chunk(n):
                xt[n] = xs_pool.tile([P, KT, QC], f16, name="xt", tag="xt")
                nc.sync.dma_start(
                    out=xt[n],
                    in_=xT[0:D_MODEL, ts(n, QC)].rearrange(
                        "(k p) q -> p k q", p=P))

            for k in range(KT):
                load_wa_cols(k, 1)
            for k in range(KT):
                load_wa_cols(k, 2)
            nc.sync.dma_start(out=tri_sb, in_=tri[:])
            nc.sync.dma_start(out=idn_sb, in_=idn[:])
            load_x_chunk(1)
            for j in range(PAIRS):
                nc.sync.dma_start(out=wp_sb[j], in_=wp[ts(j, P), :])
            if has_qk_bias:
                nc.sync.dma_start(out=xrow, in_=xT[D_MODEL:D_MODEL + 1, :])
                nc.sync.dma_start(out=wrow, in_=wa[D_MODEL:D_MODEL + 1, :])

            # ---------------- qkv projection helpers ----------------
            # w block col ranges: block 2p -> q pair p, 2p+1 -> k pair p,
            # block 8+j -> v (cols 1024 + j*128)
            def qk_copy_dest(blk, n):
                p, kind = divmod(blk, 2)
                return (qT if kind == 0 else kT)[p][n]

            def emit_qk_block(n, blk, ps):
                """Accumulate w-block x x-chunk into ps and copy to SBUF."""
                for k in range(KT):
                    nc.tensor.matmul(
                        ps[:], wa_sb[k // 2][:, k % 2, ts(blk, P)],
                        xt[n][:, k, :],
                        start=(k == 0),
                        stop=(k == KT - 1) and not has_qk_bias)
                if has_qk_bias:
                    nc.tensor.matmul(
                        ps[:], wrow[:, ts(blk, P)], xrow[:, ts(n, QC)],
                        start=False, stop=True)
                nc.vector.tensor_copy(out=qk_copy_dest(blk, n), in_=ps[:])

            def emit_v_block(n, tsub, ps):
                tt = 4 * n + tsub
                for k in range(KT):
                    nc.tensor.matmul(
                        ps[:], xt[n][:, k, ts(tsub, P)],
                        wa_sb[k // 2][:, k % 2, 1024:1536],
                        start=(k == 0),
                        stop=(k == KT - 1) and not has_qk_bias)
                if has_qk_bias:
                    nc.tensor.matmul(
                        ps[:], xrow[:, n * QC + tsub * P:
                                    n * QC + (tsub + 1) * P],
                        wrow[:, 1024:1536], start=False, stop=True)
                nc.vector.memset(v_sb[tt][:, :, 64:65], 1.0)
                nc.vector.tensor_copy(
                    out=v_sb[tt][:, :, 0:64],
                    in_=ps.rearrange("p (h c) -> p h c", c=64))

            def qkv_chunk_thunks(n):
                """Filler thunks for chunk n (needs xt[n] loaded)."""
                thunks = []
                for blk in range(8):
                    def t(blk=blk):
                        ps = lin_pool.tile([P, QC], f32, name="lps",
                                           tag="lps")
                        emit_qk_block(n, blk, ps)
                    thunks.append((t, 8 * QC * _PE_PER_ROW))
                for tsub in range(4):
                    def t(tsub=tsub):
                        ps = lin_pool.tile([P, QC], f32, name="lps",
                                           tag="lps")
                        emit_v_block(n, tsub, ps)
                    thunks.append((t, 8 * QC * _PE_PER_ROW))
                return thunks

            # ---------------- chunk 0: k-outer for early start ----------
            # 3 passes of 4 accumulators (2 lin tiles + 2 halves of an st
            # tile) so compute streams while wa/x DMAs land.
            for pass_blocks in ([0, 1, 2, 3], [4, 5, 6, 7],
                                [(0, 0), (0, 1), (0, 2), (0, 3)]):
                l0 = lin_pool.tile([P, QC], f32, name="lps", tag="lps")
                l1 = lin_pool.tile([P, QC], f32, name="lps", tag="lps")
                s0 = st_pool.tile([P, 2, QC], f32, name="st", tag="st")
                accs = [l0, l1, s0[:, 0, :], s0[:, 1, :]]
                is_v = isinstance(pass_blocks[0], tuple)
                for k in range(KT):
                    for a, blk in zip(accs, pass_blocks):
                        if is_v:
                            _, tsub = blk
                            nc.tensor.matmul(
                                a[:], xt[0][:, k, ts(tsub, P)],
                                wa_sb[k // 2][:, k % 2, 1024:1536],
                                start=(k == 0),
                                stop=(k == KT - 1) and not has_qk_bias)
                        else:
                            nc.tensor.matmul(
                                a[:], wa_sb[k // 2][:, k % 2, ts(blk, P)],
                                xt[0][:, k, :],
                                start=(k == 0),
                                stop=(k == KT - 1) and not has_qk_bias)
                for a, blk in zip(accs, pass_blocks):
                    if is_v:
                        _, tsub = blk
                        if has_qk_bias:
                            nc.tensor.matmul(
                                a[:], xrow[:, ts(tsub, P)],
                                wrow[:, 1024:1536], start=False, stop=True)
                        nc.vector.memset(v_sb[tsub][:, :, 64:65], 1.0)
                        nc.vector.tensor_copy(
                            out=v_sb[tsub][:, :, 0:64],
                            in_=a.rearrange("p (h c) -> p h c", c=64))
                    else:
                        if has_qk_bias:
                            nc.tensor.matmul(
                                a[:], wrow[:, ts(blk, P)], xrow[:, ts(0, QC)],
                                start=False, stop=True)
                        nc.vector.tensor_copy(out=qk_copy_dest(blk, 0),
                                              in_=a[:])

            # ---------------- attention ----------------
            def attn_chunk(c, pair_list, fillers, debt):
                """Attention for q-chunk c over pairs in pair_list.

                Software-pipelined: PV for (p, h) is emitted two h-slots
                after its QK groups, so the following pair's QK matmuls
                cover the exp latency. fillers: deque of (thunk, pe_ns);
                popped when the act-debt (exp time not covered by
                attention PE work) exceeds one thunk's worth.
                """
                # groups: (kt_a, kt_b, so_a, so_b)
                groups = [(2 * g, 2 * g + 1, 0, 0) for g in range(2 * c)]
                groups.append((4 * c, 4 * c + 1, 0, P))
                groups.append((4 * c + 2, 4 * c + 3, 2 * P, 3 * P))
                last_p = pair_list[-1]

                def pops():
                    while fillers and debt[0] > fillers[0][1]:
                        t, tns = fillers.popleft()
                        t()
                        debt[0] -= tns

                ysub_map = {}

                def emit_qk(p, h):
                    hb = h * 64
                    pieces = []
                    for ka, kb, soa, sob in groups:
                        st = st_pool.tile([P, 2, QC], f32, name="st",
                                          tag="st")
                        ex = ex_pool.tile([P, 2, QC], f16, name="ex",
                                          tag="ex")
                        for j, (kt, so) in enumerate(((ka, soa),
                                                      (kb, sob))):
                            nc.tensor.matmul(
                                st[:, j, so:QC],
                                kT[p][kt // 4][hb:hb + 64, ts(kt % 4, P)],
                                qT[p][c][hb:hb + 64, so:QC],
                                start=True, stop=True)
                            pieces.append((kt, so, ex, j))
                        # exp: one op per group over [min_so:512] of both
                        # pieces; the [soa:sob) part of piece b is unwritten
                        # PSUM whose exp is never read by PV
                        mso = min(soa, sob)
                        nc.scalar.activation(ex[:, :, mso:QC],
                                             st[:, :, mso:QC], EXPF,
                                             scale=0.125)
                        act_ns = (2 * (QC - mso) * _ACT_PER_ELEM
                                  + _ACT_PER_OP)
                        # mask the 128-wide diagonal transition blocks
                        pe_ns = 0.0
                        for j, (kt, so) in enumerate(((ka, soa),
                                                      (kb, sob))):
                            if kt >= 4 * c:
                                nc.vector.tensor_mul(
                                    ex[:, j, so:so + P],
                                    ex[:, j, so:so + P],
                                    tri_sb[:])
                            pe_ns += (QC - so) * _PE_PER_ROW
                        debt[0] += act_ns - pe_ns
                        pops()
                    return pieces

                def emit_pv(p, h, pieces):
                    lh = 2 * p + h
                    y_ps = y_pool.tile([P, 4, P], f32, name="yps",
                                       tag="yps")
                    for qs in range(4):
                        last_kt = 4 * c + qs
                        for kt, so, ex, j in pieces:
                            if kt > last_kt:
                                continue
                            jj = kt - 4 * c
                            if jj >= 0 and qs < jj:
                                continue
                            nc.tensor.matmul(
                                y_ps[:, qs, 0:65],
                                ex[:, j, ts(qs, P)],
                                v_sb[kt][:, lh, :],
                                start=(kt == 0),
                                stop=(kt == last_kt))
                            debt[0] -= 65 * _PE_PER_ROW
                            pops()
                    # normalize: recip of denominators, scale 64 v-dims
                    r = r_pool.tile([P, 4], f32, name="rcp", tag="rcp")
                    nc.vector.reciprocal(out=r[:], in_=y_ps[:, :, 64:65])
                    dst = ysub_map[p]
                    for qs in range(4):
                        nc.vector.tensor_scalar_mul(
                            out=dst[qs][:, h, :],
                            in0=y_ps[:, qs, 0:64],
                            scalar1=r[:, qs:qs + 1])
                    if h == 1:
                        # transpose y [128 q, 128 dims] -> yT [dims, q]
                        if c == 3 and p == last_p:
                            # tail path: f32 PE transpose through a free st
                            # PSUM slice + DVE copy (avoids 4 serialized
                            # HWDGE ops right before the proj(3) closes)
                            tp = st_pool.tile([P, 2, QC], f32, name="st",
                                              tag="st")
                            for qs in range(4):
                                nc.tensor.transpose(
                                    tp[:, qs % 2, 0:P], dst[qs][:],
                                    idn_sb[:])
                                nc.vector.tensor_copy(
                                    out=yT[p][c][:, ts(qs, P)],
                                    in_=tp[:, qs % 2, 0:P])
                        else:
                            for qs in range(4):
                                nc.sync.dma_start(
                                    out=yT[p][c][:, ts(qs, P)],
                                    in_=dst[qs][:], transpose=True)

                pending = collections.deque()
                for p in pair_list:
                    if c == 3 and p == last_p:
                        ysub_map[p] = [
                            ysb_pool.tile([P, 2, 64], f32, name="ysbf",
                                          tag="ysbf") for _ in range(4)]
                    else:
                        ysub_map[p] = [
                            ysb_pool.tile([P, 2, 64], f16, name="ysb",
                                          tag="ysb") for _ in range(4)]
                    for h in (0, 1):
                        pending.append((p, h, emit_qk(p, h)))
                        if len(pending) == 4:
                            emit_pv(*pending.popleft())
                while pending:
                    emit_pv(*pending.popleft())

            def proj_thunks(c, split_dma=False):
                """Partial out-proj for T-tiles of chunk c (all pairs)."""
                thunks = []
                osb = {}

                def mk(tt, half):
                    def t():
                        if half == 0 and not split_dma:
                            osb[tt] = o_pool.tile([P, 1024], f16, name="osb",
                                                  tag="osb")
                        ps = lin_pool.tile([P, QC], f32, name="lps",
                                           tag="lps")
                        for p in range(PAIRS):
                            nc.tensor.matmul(
                                ps[:], yT[p][tt // 4][:, ts(tt % 4, P)],
                                wp_sb[p][:, ts(half, QC)],
                                start=(p == 0), stop=(p == PAIRS - 1))
                        if split_dma:
                            ob = o_pool.tile([P, 1024], f16, name="osb",
                                             tag="osb")
                            nc.vector.tensor_copy(out=ob[:, 0:QC], in_=ps[:])
                            nc.sync.dma_start(
                                out=out[ts(tt, P), ts(half, QC)],
                                in_=ob[:, 0:QC])
                        else:
                            nc.vector.tensor_copy(
                                out=osb[tt][:, ts(half, QC)], in_=ps[:])
                            if half == 1:
                                nc.sync.dma_start(out=out[ts(tt, P), :],
                                                  in_=osb[tt][:])
                    return t

                for tt in range(4 * c, 4 * c + 4):
                    for half in (0, 1):
                        thunks.append((mk(tt, half),
                                       PAIRS * QC * _PE_PER_ROW))
                return thunks

            # ---------------- main schedule ----------------
            debt = [0.0]
            fillers = collections.deque(qkv_chunk_thunks(1))
            attn_chunk(0, range(PAIRS), fillers, debt)
            while fillers:
                fillers.popleft()[0]()

            load_x_chunk(2)
            fillers = collections.deque(qkv_chunk_thunks(2))
            attn_chunk(1, range(PAIRS), fillers, debt)
            while fillers:
                fillers.popleft()[0]()

            load_x_chunk(3)
            fillers = collections.deque(qkv_chunk_thunks(3))
            attn_chunk(2, range(PAIRS), fillers, debt)
            while fillers:
                fillers.popleft()[0]()

            # proj(3): keep two groups' pair-0..2 matmuls in flight ahead
            # of the pair-3 close so the last transposes are hidden
            open_g = collections.deque()

            def open_group(tt, half):
                ps = lin_pool.tile([P, QC], f32, name="lps", tag="lps")
                for p in range(PAIRS - 1):
                    nc.tensor.matmul(
                        ps[:], yT[p][tt // 4][:, ts(tt % 4, P)],
                        wp_sb[p][:, ts(half, QC)],
                        start=(p == 0), stop=False)
                open_g.append((tt, half, ps))

            osb3 = {}

            def close_group():
                tt, half, ps = open_g.popleft()
                nc.tensor.matmul(
                    ps[:], yT[PAIRS - 1][tt // 4][:, ts(tt % 4, P)],
                    wp_sb[PAIRS - 1][:, ts(half, QC)],
                    start=False, stop=True)
                if half == 0:
                    osb3[tt] = o_pool.tile([P, 1024], f16, name="osb",
                                           tag="osb")
                nc.vector.tensor_copy(out=osb3[tt][:, ts(half, QC)],
                                      in_=ps[:])
                if tt == 15:
                    # final tile: DMA each half as soon as it is copied
                    nc.sync.dma_start(out=out[ts(tt, P), ts(half, QC)],
                                      in_=osb3[tt][:, ts(half, QC)])
                elif half == 1:
                    nc.sync.dma_start(out=out[ts(tt, P), :],
                                      in_=osb3[tt][:])

            fillers = collections.deque(
                proj_thunks(0) + proj_thunks(1) + proj_thunks(2))
            g3 = [(tt, half) for tt in range(12, 16) for half in (0, 1)]
            for tt, half in g3[:2]:
                fillers.append(
                    (lambda tt=tt, half=half: open_group(tt, half),
                     (PAIRS - 1) * QC * _PE_PER_ROW))
            attn_chunk(3, range(PAIRS), fillers, debt)
            while fillers:
                fillers.popleft()[0]()

            for tt, half in g3[2:]:
                close_group()
                open_group(tt, half)
            while open_g:
                close_group()

    nc.compile()
    return nc


def _make_runner(nc):
    """Reusable 8-core SPMD runner (jit built once)."""
    import jax
    from jax.sharding import Mesh, PartitionSpec
    from jax.experimental.shard_map import shard_map
    from concourse import bass2jax
    import concourse.mybir as mybir

    bass2jax.install_neuronx_cc_hook()
    partition_name = (nc.partition_id_tensor.name
                      if nc.partition_id_tensor else None)
    in_names, out_names, out_avals, zero_outs = [], [], [], []
    for alloc in nc.m.functions[0].allocations:
        if not isinstance(alloc, mybir.MemoryLocationSet):
            continue
        name = alloc.memorylocations[0].name
        if alloc.kind == "ExternalInput":
            if name != partition_name:
                in_names.append(name)
        elif alloc.kind == "ExternalOutput":
            shape = tuple(alloc.tensor_shape)
            dtype = mybir.dt.np(alloc.dtype)
            out_names.append(name)
            out_avals.append(jax.core.ShapedArray(shape, dtype))
            zero_outs.append(np.zeros(shape, dtype))
    n_params = len(in_names)
    n_outs = len(out_avals)
    all_in = list(in_names) + list(out_names)
    if partition_name is not None:
        all_in.append(partition_name)

    def _body(*args):
        operands = list(args)
        if partition_name is not None:
            operands.append(bass2jax.partition_id_tensor())
        outs = bass2jax._bass_exec_p.bind(
            *operands,
            out_avals=tuple(out_avals),
            in_names=tuple(all_in),
            out_names=tuple(out_names),
            lowering_input_output_aliases=(),
            sim_require_finite=True,
            sim_require_nnan=True,
            nc=nc,
        )
        return tuple(outs)

    devices = jax.devices()[:N_CORES]
    mesh = Mesh(np.asarray(devices), ("core",))
    in_specs = (PartitionSpec("core"),) * (n_params + n_outs)
    out_specs = (PartitionSpec("core"),) * n_outs
    donate = tuple(range(n_params, n_params + n_outs))
    sharded = jax.jit(
        shard_map(_body, mesh=mesh, in_specs=in_specs, out_specs=out_specs,
                  check_rep=False),
        donate_argnums=donate, keep_unused=True)

    def run(in_maps):
        per_core = [[np.asarray(m[k]) for k in in_names] for m in in_maps]
        concat_in = [
            np.concatenate([per_core[c][i] for c in range(N_CORES)], axis=0)
            for i in range(n_params)]
        concat_zeros = [
            np.zeros((N_CORES * z.shape[0], *z.shape[1:]), z.dtype)
            for z in zero_outs]
        outs = sharded(*concat_in, *concat_zeros)
        jax.block_until_ready(outs)
        return [
            {name: np.asarray(outs[i]).reshape(N_CORES, *out_avals[i].shape)[c]
             for i, name in enumerate(out_names)}
            for c in range(N_CORES)]

    return run


def kernel(x, w_qkv, b_qkv, w_proj, b_proj):
    x = np.asarray(x, dtype=np.float32)
    w_qkv = np.asarray(w_qkv, dtype=np.float32)
    b_qkv = np.asarray(b_qkv, dtype=np.float32)
    w_proj = np.asarray(w_proj, dtype=np.float32)
    b_proj = np.asarray(b_proj, dtype=np.float32)

    w_q, w_k, w_v = w_qkv[0:1024], w_qkv[1024:2048], w_qkv[2048:3072]
    b_q, b_k, b_v = b_qkv[0:1024], b_qkv[1024:2048], b_qkv[2048:3072]
    has_qk_bias = bool(np.any(b_q) or np.any(b_k))

    key = ("runner", has_qk_bias)
    if key not in _RUNNER_CACHE:
        nc = _build(has_qk_bias)
        _RUNNER_CACHE[key] = _make_runner(nc)
    run = _RUNNER_CACHE[key]

    # causal transition-block mask: tri[k, i] = 1.0 iff k <= i
    kk = np.arange(P)
    tri = (kk[:, None] <= kk[None, :]).astype(np.float16)
    idn = np.eye(P, dtype=np.float32)

    in_maps = []
    for core in range(N_CORES):
        b, g = divmod(core, 2)
        xT_c = x[b].T.astype(np.float16)  # [1024, 2048]
        if has_qk_bias:
            xT_c = np.concatenate(
                [xT_c, np.ones((1, T), np.float16)], axis=0)
        KD = D_MODEL + (1 if has_qk_bias else 0)
        wa_c = np.empty((KD, W_COLS), np.float32)
        wp_c = np.empty((512, 1024), np.float32)
        for p in range(PAIRS):
            hA = 8 * g + 2 * p
            hB = hA + 1
            cols = p * 256
            wa_c[:D_MODEL, cols + 0:cols + 64] = w_q[hA * 64:(hA + 1) * 64].T
            wa_c[:D_MODEL, cols + 64:cols + 128] = w_q[hB * 64:(hB + 1) * 64].T
            wa_c[:D_MODEL, cols + 128:cols + 192] = w_k[hA * 64:(hA + 1) * 64].T
            wa_c[:D_MODEL, cols + 192:cols + 256] = w_k[hB * 64:(hB + 1) * 64].T
            if has_qk_bias:
                wa_c[D_MODEL, cols + 0:cols + 64] = b_q[hA * 64:(hA + 1) * 64]
                wa_c[D_MODEL, cols + 64:cols + 128] = b_q[hB * 64:(hB + 1) * 64]
                wa_c[D_MODEL, cols + 128:cols + 192] = b_k[hA * 64:(hA + 1) * 64]
                wa_c[D_MODEL, cols + 192:cols + 256] = b_k[hB * 64:(hB + 1) * 64]
            # wp rows pair-major: [hA dims 0..63 | hB dims 64..127]
            wp_c[p * 128:p * 128 + 64, :] = w_proj.T[hA * 64:(hA + 1) * 64, :]
            wp_c[p * 128 + 64:p * 128 + 128, :] = \
                w_proj.T[hB * 64:(hB + 1) * 64, :]
        # v columns, head-major for the group
        for lh in range(8):
            head = 8 * g + lh
            wa_c[:D_MODEL, 1024 + lh * 64:1024 + (lh + 1) * 64] = \
                w_v[head * 64:(head + 1) * 64].T
            if has_qk_bias:
                wa_c[D_MODEL, 1024 + lh * 64:1024 + (lh + 1) * 64] = \
                    b_v[head * 64:(head + 1) * 64]
        in_maps.append({
            "xT": xT_c,
            "wa": wa_c.astype(np.float16),
            "wp": wp_c.astype(np.float16),
            "tri": tri,
            "idn": idn,
        })

    results = run(in_maps)

    # partial-sum unshard: the two head-group cores of each batch each
    # produced out_partial[T, 1024]; add them.
    out = np.empty((B, T, D_MODEL), dtype=np.float32)
    for b in range(B):
        out[b] = (results[2 * b]["out"].astype(np.float32)
                  + results[2 * b + 1]["out"].astype(np.float32))

    # exact host-side bias folds (v-bias rides softmax row-sums == 1 and is
    # on-device in the qk-bias build; proj bias is additive)
    if np.any(b_v) and not has_qk_bias:
        out += (b_v @ w_proj.T)[None, None, :]
    if np.any(b_proj):
        out += b_proj[None, None, :]
    return out


# revision 6
# speedup vs baseline: 1.0402x; 1.0004x over previous
"""Causal self-attention (B=4, T=2048, C=1024, H=16) on 8 TRN2 NeuronCores.

Sharding: core = 2*b + g (b = batch 0..3, g = head-group 0..1). Each core
computes qkv + attention for its batch and its 8 heads, then a PARTIAL
output projection over its own 512 y-dims for ALL 1024 output columns.
The host adds the two partials per batch (partial-sum unsharding) - no
device collectives at all.

All matmuls run in fp16 (1 PE cycle/row, no min-width constraint),
accumulation in fp32 PSUM. Attention PV uses the cheap orientation
out[q,65] = ex^T @ [V | 1] (65 rows per (k-tile, q-subtile) instead of
512), with the softmax denominator from the ones column; y is normalized
with a per-partition scalar multiply and transposed back to [dims, T]
with the DMA XBAR transpose (off the PE).

The attention inner loop is Activation-engine paced (exp); qkv-proj and
out-proj matmul "filler" work is interleaved between score/PV groups via
a debt counter so the PE never idles waiting for exp.
"""
import collections
import numpy as np

D_MODEL = 1024
N_HEAD = 16
D_HEAD = 64
B = 4
T = 2048
N_CORES = 8
P = 128
PAIRS = 4          # head pairs per core
NQ = 4             # q-chunks of 512
QC = 512           # q chunk width
KT = D_MODEL // P  # 8 contraction tiles for the qkv projection
W_COLS = 1536      # 1024 q/k cols + 512 v cols per core

_RUNNER_CACHE = {}

# cost-model-ish estimates (ns) for the act-debt interleaver
_ACT_PER_ELEM = 0.833
_ACT_PER_OP = 185.0
_PE_PER_ROW = 0.4167


def _build(has_qk_bias: bool, _nphases: int = 5):
    from concourse import bacc
    import concourse.mybir as mybir
    from concourse.tile import TileContext
    from concourse.bass import ts

    f32 = mybir.dt.float32
    f16 = mybir.dt.float16
    KD = D_MODEL + (1 if has_qk_bias else 0)

    nc = bacc.Bacc("TRN2", target_bir_lowering=False, debug=False,
                   num_devices=N_CORES)
    xT = nc.dram_tensor("xT", [KD, T], f16, kind="ExternalInput")
    wa = nc.dram_tensor("wa", [KD, W_COLS], f16, kind="ExternalInput")
    wp = nc.dram_tensor("wp", [512, 1024], f16, kind="ExternalInput")
    tri = nc.dram_tensor("tri", [P, P], f16, kind="ExternalInput")
    idn = nc.dram_tensor("idn", [P, P], f32, kind="ExternalInput")
    out = nc.dram_tensor("out", [T, 1024], f16, kind="ExternalOutput")

    EXPF = mybir.ActivationFunctionType.Exp

    with TileContext(nc) as tc:
        with (
            tc.tile_pool(name="wts", bufs=1) as wts,
            tc.tile_pool(name="qk_res", bufs=1) as qk_res,
            tc.tile_pool(name="v_res", bufs=1) as v_res,
            tc.tile_pool(name="yt_res", bufs=1) as yt_res,
            tc.tile_pool(name="xs", bufs=2) as xs_pool,
            tc.tile_pool(name="exp", bufs=34) as ex_pool,
            tc.tile_pool(name="ysb", bufs=8) as ysb_pool,
            tc.tile_pool(name="rcp", bufs=4) as r_pool,
            tc.tile_pool(name="osb", bufs=4) as o_pool,
            tc.tile_pool(name="lin", bufs=2, space="PSUM") as lin_pool,
            tc.tile_pool(name="st", bufs=2, space="PSUM") as st_pool,
            tc.tile_pool(name="yps", bufs=2, space="PSUM") as y_pool,
        ):
            # ---------------- persistent tiles ----------------
            # wa_sb[i] holds contraction k-tiles 2i, 2i+1: [128, 2, 1536]
            wa_sb = [wts.tile([P, 2, W_COLS], f16, name=f"wa{i}")
                     for i in range(4)]
            wp_sb = [wts.tile([P, 1024], f16, name=f"wp{j}")
                     for j in range(PAIRS)]
            tri_sb = wts.tile([P, P], f16, name="tri_sb")
            idn_sb = wts.tile([P, P], f32, name="idn_sb")
            if has_qk_bias:
                xrow = wts.tile([1, T], f16, name="xrow")
                wrow = wts.tile([1, W_COLS], f16, name="wrow")
            # qT[p][c]/kT[p][c]: [128 dims (2 heads x 64), 512 T]
            qT = [[qk_res.tile([P, QC], f16, name=f"qT{p}_{c}")
                   for c in range(NQ)] for p in range(PAIRS)]
            kT = [[qk_res.tile([P, QC], f16, name=f"kT{p}_{c}")
                   for c in range(NQ)] for p in range(PAIRS)]
            # v_sb[tt]: [128 keys, 8 heads, 65 (v | 1)]
            v_sb = [v_res.tile([P, 8, 65], f16, name=f"v{t}")
                    for t in range(T // P)]
            # yT[p][c]: [128 dims, 512 T] (normalized, transposed)
            yT = [[yt_res.tile([P, QC], f16, name=f"yT{p}_{c}")
                   for c in range(NQ)] for p in range(PAIRS)]

            # ---------------- DMA loads ----------------
            # x chunk tiles; chunk 0 split in 4 pieces for early start
            xt = [None] * NQ
            xt[0] = xs_pool.tile([P, KT, QC], f16, name="xt", tag="xt")

            def load_x_piece(i):
                nc.sync.dma_start(
                    out=xt[0][:, 2 * i:2 * i + 2, :],
                    in_=xT[ts(i, 2 * P), ts(0, QC)].rearrange(
                        "(j p) q -> p j q", p=P))

            def load_wa(i, j):
                nc.sync.dma_start(
                    out=wa_sb[i][:, j, :],
                    in_=wa[ts(2 * i + j, P), :])

            # chunk-0 weight loads arrive in column-waves matching the
            # k-outer passes: wave 1 = qk cols 0:512 (pass A), wave 2 =
            # cols 512:1024 (pass B), wave 3 = v cols (pass C)
            def load_wa_cols(k, cc):
                nc.sync.dma_start(
                    out=wa_sb[k // 2][:, k % 2, ts(cc, QC)],
                    in_=wa[ts(k, P), ts(cc, QC)])

            load_wa_cols(0, 0)
            for i in range(4):
                load_x_piece(i)
                if 2 * i + 1 < KT:
                    load_wa_cols(2 * i + 1, 0)
                if 2 * i + 2 < KT:
                    load_wa_cols(2 * i + 2, 0)
            def load_x_chunk(n):
                xt[n] = xs_pool.tile([P, KT, QC], f16, name="xt", tag="xt")
                nc.sync.dma_start(
                    out=xt[n],
                    in_=xT[0:D_MODEL, ts(n, QC)].rearrange(
                        "(k p) q -> p k q", p=P))

            for k in range(KT):
                load_wa_cols(k, 1)
            for k in range(KT):
                load_wa_cols(k, 2)
            nc.sync.dma_start(out=tri_sb, in_=tri[:])
            nc.sync.dma_start(out=idn_sb, in_=idn[:])
            load_x_chunk(1)
            for j in range(PAIRS):
                nc.sync.dma_start(out=wp_sb[j], in_=wp[ts(j, P), :])
            if has_qk_bias:
                nc.sync.dma_start(out=xrow, in_=xT[D_MODEL:D_MODEL + 1, :])
                nc.sync.dma_start(out=wrow, in_=wa[D_MODEL:D_MODEL + 1, :])

            # ---------------- qkv projection helpers ----------------
            # w block col ranges: block 2p -> q pair p, 2p+1 -> k pair p,
            # block 8+j -> v (cols 1024 + j*128)
            def qk_copy_dest(blk, n):
                p, kind = divmod(blk, 2)
                return (qT if kind == 0 else kT)[p][n]

            def emit_qk_block(n, blk, ps):
                """Accumulate w-block x x-chunk into ps and copy to SBUF."""
                for k in range(KT):
                    nc.tensor.matmul(
                        ps[:], wa_sb[k // 2][:, k % 2, ts(blk, P)],
                        xt[n][:, k, :],
                        start=(k == 0),
                        stop=(k == KT - 1) and not has_qk_bias)
                if has_qk_bias:
                    nc.tensor.matmul(
                        ps[:], wrow[:, ts(blk, P)], xrow[:, ts(n, QC)],
                        start=False, stop=True)
                nc.vector.tensor_copy(out=qk_copy_dest(blk, n), in_=ps[:])

            def emit_v_block(n, tsub, ps):
                tt = 4 * n + tsub
                for k in range(KT):
                    nc.tensor.matmul(
                        ps[:], xt[n][:, k, ts(tsub, P)],
                        wa_sb[k // 2][:, k % 2, 1024:1536],
                        start=(k == 0),
                        stop=(k == KT - 1) and not has_qk_bias)
                if has_qk_bias:
                    nc.tensor.matmul(
                        ps[:], xrow[:, n * QC + tsub * P:
                                    n * QC + (tsub + 1) * P],
                        wrow[:, 1024:1536], start=False, stop=True)
                nc.vector.memset(v_sb[tt][:, :, 64:65], 1.0)
                nc.vector.tensor_copy(
                    out=v_sb[tt][:, :, 0:64],
                    in_=ps.rearrange("p (h c) -> p h c", c=64))

            def qkv_chunk_thunks(n):
                """Filler thunks for chunk n (needs xt[n] loaded)."""
                thunks = []
                for blk in range(8):
                    def t(blk=blk):
                        ps = lin_pool.tile([P, QC], f32, name="lps",
                                           tag="lps")
                        emit_qk_block(n, blk, ps)
                    thunks.append((t, 8 * QC * _PE_PER_ROW))
                for tsub in range(4):
                    def t(tsub=tsub):
                        ps = lin_pool.tile([P, QC], f32, name="lps",
                                           tag="lps")
                        emit_v_block(n, tsub, ps)
                    thunks.append((t, 8 * QC * _PE_PER_ROW))
                return thunks

            # ---------------- chunk 0: k-outer for early start ----------
            # 3 passes of 4 accumulators (2 lin tiles + 2 halves of an st
            # tile) so compute streams while wa/x DMAs land.
            for pass_blocks in ([0, 1, 2, 3], [4, 5, 6, 7],
                                [(0, 0), (0, 1), (0, 2), (0, 3)]):
                l0 = lin_pool.tile([P, QC], f32, name="lps", tag="lps")
                l1 = lin_pool.tile([P, QC], f32, name="lps", tag="lps")
                s0 = st_pool.tile([P, 2, QC], f32, name="st", tag="st")
                accs = [l0, l1, s0[:, 0, :], s0[:, 1, :]]
                is_v = isinstance(pass_blocks[0], tuple)
                for k in range(KT):
                    for a, blk in zip(accs, pass_blocks):
                        if is_v:
                            _, tsub = blk
                            nc.tensor.matmul(
                                a[:], xt[0][:, k, ts(tsub, P)],
                                wa_sb[k // 2][:, k % 2, 1024:1536],
                                start=(k == 0),
                                stop=(k == KT - 1) and not has_qk_bias)
                        else:
                            nc.tensor.matmul(
                                a[:], wa_sb[k // 2][:, k % 2, ts(blk, P)],
                                xt[0][:, k, :],
                                start=(k == 0),
                                stop=(k == KT - 1) and not has_qk_bias)
                for a, blk in zip(accs, pass_blocks):
                    if is_v:
                        _, tsub = blk
                        if has_qk_bias:
                            nc.tensor.matmul(
                                a[:], xrow[:, ts(tsub, P)],
                                wrow[:, 1024:1536], start=False, stop=True)
                        nc.vector.memset(v_sb[tsub][:, :, 64:65], 1.0)
                        nc.vector.tensor_copy(
                            out=v_sb[tsub][:, :, 0:64],
                            in_=a.rearrange("p (h c) -> p h c", c=64))
                    else:
                        if has_qk_bias:
                            nc.tensor.matmul(
                                a[:], wrow[:, ts(blk, P)], xrow[:, ts(0, QC)],
                                start=False, stop=True)
                        nc.vector.tensor_copy(out=qk_copy_dest(blk, 0),
                                              in_=a[:])

            # ---------------- attention ----------------
            def attn_chunk(c, pair_list, fillers, debt):
                """Attention for q-chunk c over pairs in pair_list.

                Software-pipelined: PV for (p, h) is emitted two h-slots
                after its QK groups, so the following pair's QK matmuls
                cover the exp latency. fillers: deque of (thunk, pe_ns);
                popped when the act-debt (exp time not covered by
                attention PE work) exceeds one thunk's worth.
                """
                # groups: (kt_a, kt_b, so_a, so_b)
                groups = [(2 * g, 2 * g + 1, 0, 0) for g in range(2 * c)]
                groups.append((4 * c, 4 * c + 1, 0, P))
                groups.append((4 * c + 2, 4 * c + 3, 2 * P, 3 * P))
                last_p = pair_list[-1]

                def pops():
                    while fillers and debt[0] > fillers[0][1]:
                        t, tns = fillers.popleft()
                        t()
                        debt[0] -= tns

                ysub_map = {}

                def emit_qk(p, h):
                    hb = h * 64
                    pieces = []
                    for ka, kb, soa, sob in groups:
                        st = st_pool.tile([P, 2, QC], f32, name="st",
                                          tag="st")
                        ex = ex_pool.tile([P, 2, QC], f16, name="ex",
                                          tag="ex")
                        for j, (kt, so) in enumerate(((ka, soa),
                                                      (kb, sob))):
                            nc.tensor.matmul(
                                st[:, j, so:QC],
                                kT[p][kt // 4][hb:hb + 64, ts(kt % 4, P)],
                                qT[p][c][hb:hb + 64, so:QC],
                                start=True, stop=True)
                            pieces.append((kt, so, ex, j))
                        # exp: one op per group over [min_so:512] of both
                        # pieces; the [soa:sob) part of piece b is unwritten
                        # PSUM whose exp is never read by PV
                        mso = min(soa, sob)
                        nc.scalar.activation(ex[:, :, mso:QC],
                                             st[:, :, mso:QC], EXPF,
                                             scale=0.125)
                        act_ns = (2 * (QC - mso) * _ACT_PER_ELEM
                                  + _ACT_PER_OP)
                        # mask the 128-wide diagonal transition blocks
                        pe_ns = 0.0
                        for j, (kt, so) in enumerate(((ka, soa),
                                                      (kb, sob))):
                            if kt >= 4 * c:
                                nc.vector.tensor_mul(
                                    ex[:, j, so:so + P],
                                    ex[:, j, so:so + P],
                                    tri_sb[:])
                            pe_ns += (QC - so) * _PE_PER_ROW
                        debt[0] += act_ns - pe_ns
                        pops()
                    return pieces

                def emit_pv(p, h, pieces):
                    lh = 2 * p + h
                    dst = ysub_map[p]
                    finalize = (c == 3 and p == last_p and h == 1)
                    y_ps = y_pool.tile([P, 4, P], f32, name="yps",
                                       tag="yps")
                    if finalize:
                        tp = st_pool.tile([P, 2, QC], f32, name="st",
                                          tag="st")

                    def fin_transpose(qs):
                        nc.tensor.transpose(tp[:, qs % 2, 0:P],
                                            dst[qs][:], idn_sb[:])
                        nc.vector.tensor_copy(
                            out=yT[p][c][:, ts(qs, P)],
                            in_=tp[:, qs % 2, 0:P])

                    for qs in range(4):
                        last_kt = 4 * c + qs
                        for kt, so, ex, j in pieces:
                            if kt > last_kt:
                                continue
                            jj = kt - 4 * c
                            if jj >= 0 and qs < jj:
                                continue
                            nc.tensor.matmul(
                                y_ps[:, qs, 0:65],
                                ex[:, j, ts(qs, P)],
                                v_sb[kt][:, lh, :],
                                start=(kt == 0),
                                stop=(kt == last_kt))
                            debt[0] -= 65 * _PE_PER_ROW
                            pops()
                        if finalize:
                            # per-qs normalize then PE transpose of the
                            # previous qs, pipelining the tail chain
                            rq = r_pool.tile([P, 4], f32, name="rcp",
                                             tag="rcp")
                            nc.vector.reciprocal(
                                out=rq[:, 0:1], in_=y_ps[:, qs, 64:65])
                            nc.vector.tensor_scalar_mul(
                                out=dst[qs][:, h, :],
                                in0=y_ps[:, qs, 0:64],
                                scalar1=rq[:, 0:1])
                            if qs >= 1:
                                fin_transpose(qs - 1)
                    if finalize:
                        fin_transpose(3)
                        return
                    # normalize: recip of denominators, scale 64 v-dims
                    r = r_pool.tile([P, 4], f32, name="rcp", tag="rcp")
                    nc.vector.reciprocal(out=r[:], in_=y_ps[:, :, 64:65])
                    for qs in range(4):
                        nc.vector.tensor_scalar_mul(
                            out=dst[qs][:, h, :],
                            in0=y_ps[:, qs, 0:64],
                            scalar1=r[:, qs:qs + 1])
                    if h == 1:
                        # transpose y [128 q, 128 dims] -> yT [dims, q]
                        for qs in range(4):
                            nc.sync.dma_start(
                                out=yT[p][c][:, ts(qs, P)],
                                in_=dst[qs][:], transpose=True)

                pending = collections.deque()
                for p in pair_list:
                    if c == 3 and p == last_p:
                        ysub_map[p] = [
                            ysb_pool.tile([P, 2, 64], f32, name="ysbf",
                                          tag="ysbf") for _ in range(4)]
                    else:
                        ysub_map[p] = [
                            ysb_pool.tile([P, 2, 64], f16, name="ysb",
                                          tag="ysb") for _ in range(4)]
                    for h in (0, 1):
                        pending.append((p, h, emit_qk(p, h)))
                        if len(pending) == 4:
                            emit_pv(*pending.popleft())
                while pending:
                    emit_pv(*pending.popleft())

            def proj_thunks(c, split_dma=False):
                """Partial out-proj for T-tiles of chunk c (all pairs)."""
                thunks = []
                osb = {}

                def mk(tt, half):
                    def t():
                        if half == 0 and not split_dma:
                            osb[tt] = o_pool.tile([P, 1024], f16, name="osb",
                                                  tag="osb")
                        ps = lin_pool.tile([P, QC], f32, name="lps",
                                           tag="lps")
                        for p in range(PAIRS):
                            nc.tensor.matmul(
                                ps[:], yT[p][tt // 4][:, ts(tt % 4, P)],
                                wp_sb[p][:, ts(half, QC)],
                                start=(p == 0), stop=(p == PAIRS - 1))
                        if split_dma:
                            ob = o_pool.tile([P, 1024], f16, name="osb",
                                             tag="osb")
                            nc.vector.tensor_copy(out=ob[:, 0:QC], in_=ps[:])
                            nc.sync.dma_start(
                                out=out[ts(tt, P), ts(half, QC)],
                                in_=ob[:, 0:QC])
                        else:
                            nc.vector.tensor_copy(
                                out=osb[tt][:, ts(half, QC)], in_=ps[:])
                            if half == 1:
                                nc.sync.dma_start(out=out[ts(tt, P), :],
                                                  in_=osb[tt][:])
                    return t

                for tt in range(4 * c, 4 * c + 4):
                    for half in (0, 1):
                        thunks.append((mk(tt, half),
                                       PAIRS * QC * _PE_PER_ROW))
                return thunks

            # ---------------- main schedule ----------------
            debt = [0.0]
            fillers = collections.deque(qkv_chunk_thunks(1))
            attn_chunk(0, range(PAIRS), fillers, debt)
            while fillers:
                fillers.popleft()[0]()

            load_x_chunk(2)
            fillers = collections.deque(qkv_chunk_thunks(2))
            attn_chunk(1, range(PAIRS), fillers, debt)
            while fillers:
                fillers.popleft()[0]()

            load_x_chunk(3)
            fillers = collections.deque(qkv_chunk_thunks(3))
            attn_chunk(2, range(PAIRS), fillers, debt)
            while fillers:
                fillers.popleft()[0]()

            # proj(3): keep two groups' pair-0..2 matmuls in flight ahead
            # of the pair-3 close so the last transposes are hidden
            open_g = collections.deque()

            def open_group(tt, half):
                ps = lin_pool.tile([P, QC], f32, name="lps", tag="lps")
                for p in range(PAIRS - 1):
                    nc.tensor.matmul(
                        ps[:], yT[p][tt // 4][:, ts(tt % 4, P)],
                        wp_sb[p][:, ts(half, QC)],
                        start=(p == 0), stop=False)
                open_g.append((tt, half, ps))

            osb3 = {}

            def close_group():
                tt, half, ps = open_g.popleft()
                nc.tensor.matmul(
                    ps[:], yT[PAIRS - 1][tt // 4][:, ts(tt % 4, P)],
                    wp_sb[PAIRS - 1][:, ts(half, QC)],
                    start=False, stop=True)
                if half == 0:
                    osb3[tt] = o_pool.tile([P, 1024], f16, name="osb",
                                           tag="osb")
                nc.vector.tensor_copy(out=osb3[tt][:, ts(half, QC)],
                                      in_=ps[:])
                if tt == 15:
                    # final tile: DMA each half as soon as it is copied
                    nc.sync.dma_start(out=out[ts(tt, P), ts(half, QC)],
                                      in_=osb3[tt][:, ts(half, QC)])
                elif half == 1:
                    nc.sync.dma_start(out=out[ts(tt, P), :],
                                      in_=osb3[tt][:])

            fillers = collections.deque(
                proj_thunks(0) + proj_thunks(1) + proj_thunks(2))
            g3 = [(tt, half) for tt in range(12, 16) for half in (0, 1)]
            for tt, half in g3[:2]:
                fillers.append(
                    (lambda tt=tt, half=half: open_group(tt, half),
                     (PAIRS - 1) * QC * _PE_PER_ROW))
            attn_chunk(3, range(PAIRS), fillers, debt)
            while fillers:
                fillers.popleft()[0]()

            for tt, half in g3[2:]:
                close_group()
                open_group(tt, half)
            while open_g:
                close_group()

    nc.compile()
    return nc


def _make_runner(nc):
    """Reusable 8-core SPMD runner (jit built once)."""
    import jax
    from jax.sharding import Mesh, PartitionSpec
    from jax.experimental.shard_map import shard_map
    from concourse import bass2jax
    import concourse.mybir as mybir

    bass2jax.install_neuronx_cc_hook()
    partition_name = (nc.partition_id_tensor.name
                      if nc.partition_id_tensor else None)
    in_names, out_names, out_avals, zero_outs = [], [], [], []
    for alloc in nc.m.functions[0].allocations:
        if not isinstance(alloc, mybir.MemoryLocationSet):
            continue
        name = alloc.memorylocations[0].name
        if alloc.kind == "ExternalInput":
            if name != partition_name:
                in_names.append(name)
        elif alloc.kind == "ExternalOutput":
            shape = tuple(alloc.tensor_shape)
            dtype = mybir.dt.np(alloc.dtype)
            out_names.append(name)
            out_avals.append(jax.core.ShapedArray(shape, dtype))
            zero_outs.append(np.zeros(shape, dtype))
    n_params = len(in_names)
    n_outs = len(out_avals)
    all_in = list(in_names) + list(out_names)
    if partition_name is not None:
        all_in.append(partition_name)

    def _body(*args):
        operands = list(args)
        if partition_name is not None:
            operands.append(bass2jax.partition_id_tensor())
        outs = bass2jax._bass_exec_p.bind(
            *operands,
            out_avals=tuple(out_avals),
            in_names=tuple(all_in),
            out_names=tuple(out_names),
            lowering_input_output_aliases=(),
            sim_require_finite=True,
            sim_require_nnan=True,
            nc=nc,
        )
        return tuple(outs)

    devices = jax.devices()[:N_CORES]
    mesh = Mesh(np.asarray(devices), ("core",))
    in_specs = (PartitionSpec("core"),) * (n_params + n_outs)
    out_specs = (PartitionSpec("core"),) * n_outs
    donate = tuple(range(n_params, n_params + n_outs))
    sharded = jax.jit(
        shard_map(_body, mesh=mesh, in_specs=in_specs, out_specs=out_specs,
                  check_rep=False),
        donate_argnums=donate, keep_unused=True)

    def run(in_maps):
        per_core = [[np.asarray(m[k]) for k in in_names] for m in in_maps]
        concat_in = [
            np.concatenate([per_core[c][i] for c in range(N_CORES)], axis=0)
            for i in range(n_params)]
        concat_zeros = [
            np.zeros((N_CORES * z.shape[0], *z.shape[1:]), z.dtype)
            for z in zero_outs]
        outs = sharded(*concat_in, *concat_zeros)
        jax.block_until_ready(outs)
        return [
            {name: np.asarray(outs[i]).reshape(N_CORES, *out_avals[i].shape)[c]
             for i, name in enumerate(out_names)}
            for c in range(N_CORES)]

    return run


def kernel(x, w_qkv, b_qkv, w_proj, b_proj):
    x = np.asarray(x, dtype=np.float32)
    w_qkv = np.asarray(w_qkv, dtype=np.float32)
    b_qkv = np.asarray(b_qkv, dtype=np.float32)
    w_proj = np.asarray(w_proj, dtype=np.float32)
    b_proj = np.asarray(b_proj, dtype=np.float32)

    w_q, w_k, w_v = w_qkv[0:1024], w_qkv[1024:2048], w_qkv[2048:3072]
    b_q, b_k, b_v = b_qkv[0:1024], b_qkv[1024:2048], b_qkv[2048:3072]
    has_qk_bias = bool(np.any(b_q) or np.any(b_k))

    key = ("runner", has_qk_bias)
    if key not in _RUNNER_CACHE:
        nc = _build(has_qk_bias)
        _RUNNER_CACHE[key] = _make_runner(nc)
    run = _RUNNER_CACHE[key]

    # causal transition-block mask: tri[k, i] = 1.0 iff k <= i
    kk = np.arange(P)
    tri = (kk[:, None] <= kk[None, :]).astype(np.float16)
    idn = np.eye(P, dtype=np.float32)

    in_maps = []
    for core in range(N_CORES):
        b, g = divmod(core, 2)
        xT_c = x[b].T.astype(np.float16)  # [1024, 2048]
        if has_qk_bias:
            xT_c = np.concatenate(
                [xT_c, np.ones((1, T), np.float16)], axis=0)
        KD = D_MODEL + (1 if has_qk_bias else 0)
        wa_c = np.empty((KD, W_COLS), np.float32)
        wp_c = np.empty((512, 1024), np.float32)
        for p in range(PAIRS):
            hA = 8 * g + 2 * p
            hB = hA + 1
            cols = p * 256
            wa_c[:D_MODEL, cols + 0:cols + 64] = w_q[hA * 64:(hA + 1) * 64].T
            wa_c[:D_MODEL, cols + 64:cols + 128] = w_q[hB * 64:(hB + 1) * 64].T
            wa_c[:D_MODEL, cols + 128:cols + 192] = w_k[hA * 64:(hA + 1) * 64].T
            wa_c[:D_MODEL, cols + 192:cols + 256] = w_k[hB * 64:(hB + 1) * 64].T
            if has_qk_bias:
                wa_c[D_MODEL, cols + 0:cols + 64] = b_q[hA * 64:(hA + 1) * 64]
                wa_c[D_MODEL, cols + 64:cols + 128] = b_q[hB * 64:(hB + 1) * 64]
                wa_c[D_MODEL, cols + 128:cols + 192] = b_k[hA * 64:(hA + 1) * 64]
                wa_c[D_MODEL, cols + 192:cols + 256] = b_k[hB * 64:(hB + 1) * 64]
            # wp rows pair-major: [hA dims 0..63 | hB dims 64..127]
            wp_c[p * 128:p * 128 + 64, :] = w_proj.T[hA * 64:(hA + 1) * 64, :]
            wp_c[p * 128 + 64:p * 128 + 128, :] = \
                w_proj.T[hB * 64:(hB + 1) * 64, :]
        # v columns, head-major for the group
        for lh in range(8):
            head = 8 * g + lh
            wa_c[:D_MODEL, 1024 + lh * 64:1024 + (lh + 1) * 64] = \
                w_v[head * 64:(head + 1) * 64].T
            if has_qk_bias:
                wa_c[D_MODEL, 1024 + lh * 64:1024 + (lh + 1) * 64] = \
                    b_v[head * 64:(head + 1) * 64]
        in_maps.append({
            "xT": xT_c,
            "wa": wa_c.astype(np.float16),
            "wp": wp_c.astype(np.float16),
            "tri": tri,
            "idn": idn,
        })

    results = run(in_maps)

    # partial-sum unshard: the two head-group cores of each batch each
    # produced out_partial[T, 1024]; add them.
    out = np.empty((B, T, D_MODEL), dtype=np.float32)
    for b in range(B):
        out[b] = (results[2 * b]["out"].astype(np.float32)
                  + results[2 * b + 1]["out"].astype(np.float32))

    # exact host-side bias folds (v-bias rides softmax row-sums == 1 and is
    # on-device in the qk-bias build; proj bias is additive)
    if np.any(b_v) and not has_qk_bias:
        out += (b_v @ w_proj.T)[None, None, :]
    if np.any(b_proj):
        out += b_proj[None, None, :]
    return out


# revision 7
# speedup vs baseline: 1.0443x; 1.0040x over previous
"""Causal self-attention (B=4, T=2048, C=1024, H=16) on 8 TRN2 NeuronCores.

Sharding: core = 2*b + g (b = batch 0..3, g = head-group 0..1). Each core
computes qkv + attention for its batch and its 8 heads, then a PARTIAL
output projection over its own 512 y-dims for ALL 1024 output columns.
The host adds the two partials per batch (partial-sum unsharding) - no
device collectives at all.

All matmuls run in fp16 (1 PE cycle/row, no min-width constraint),
accumulation in fp32 PSUM. Attention PV uses the cheap orientation
out[q,65] = ex^T @ [V | 1] (65 rows per (k-tile, q-subtile) instead of
512), with the softmax denominator from the ones column; y is normalized
with a per-partition scalar multiply and transposed back to [dims, T]
with the DMA XBAR transpose (off the PE).

The attention inner loop is Activation-engine paced (exp); qkv-proj and
out-proj matmul "filler" work is interleaved between score/PV groups via
a debt counter so the PE never idles waiting for exp.
"""
import collections
import numpy as np

D_MODEL = 1024
N_HEAD = 16
D_HEAD = 64
B = 4
T = 2048
N_CORES = 8
P = 128
PAIRS = 4          # head pairs per core
NQ = 4             # q-chunks of 512
QC = 512           # q chunk width
KT = D_MODEL // P  # 8 contraction tiles for the qkv projection
W_COLS = 1536      # 1024 q/k cols + 512 v cols per core

_RUNNER_CACHE = {}

# cost-model-ish estimates (ns) for the act-debt interleaver
_ACT_PER_ELEM = 0.833
_ACT_PER_OP = 185.0
_PE_PER_ROW = 0.4167


def _build(has_qk_bias: bool, _nphases: int = 5):
    from concourse import bacc
    import concourse.mybir as mybir
    from concourse.tile import TileContext
    from concourse.bass import ts

    f32 = mybir.dt.float32
    f16 = mybir.dt.float16
    KD = D_MODEL + (1 if has_qk_bias else 0)

    nc = bacc.Bacc("TRN2", target_bir_lowering=False, debug=False,
                   num_devices=N_CORES)
    xT = nc.dram_tensor("xT", [KD, T], f16, kind="ExternalInput")
    wa = nc.dram_tensor("wa", [KD, W_COLS], f16, kind="ExternalInput")
    wp = nc.dram_tensor("wp", [512, 1024], f16, kind="ExternalInput")
    tri = nc.dram_tensor("tri", [P, P], f16, kind="ExternalInput")
    idn = nc.dram_tensor("idn", [P, P], f32, kind="ExternalInput")
    out = nc.dram_tensor("out", [T, 1024], f16, kind="ExternalOutput")

    EXPF = mybir.ActivationFunctionType.Exp

    with TileContext(nc) as tc:
        with (
            tc.tile_pool(name="wts", bufs=1) as wts,
            tc.tile_pool(name="qk_res", bufs=1) as qk_res,
            tc.tile_pool(name="v_res", bufs=1) as v_res,
            tc.tile_pool(name="yt_res", bufs=1) as yt_res,
            tc.tile_pool(name="xs", bufs=2) as xs_pool,
            tc.tile_pool(name="exp", bufs=40) as ex_pool,
            tc.tile_pool(name="ysb", bufs=8) as ysb_pool,
            tc.tile_pool(name="rcp", bufs=4) as r_pool,
            tc.tile_pool(name="osb", bufs=4) as o_pool,
            tc.tile_pool(name="lin", bufs=2, space="PSUM") as lin_pool,
            tc.tile_pool(name="st", bufs=2, space="PSUM") as st_pool,
            tc.tile_pool(name="yps", bufs=2, space="PSUM") as y_pool,
        ):
            # ---------------- persistent tiles ----------------
            # wa_sb[i] holds contraction k-tiles 2i, 2i+1: [128, 2, 1536]
            wa_sb = [wts.tile([P, 2, W_COLS], f16, name=f"wa{i}")
                     for i in range(4)]
            wp_sb = [wts.tile([P, 1024], f16, name=f"wp{j}")
                     for j in range(PAIRS)]
            tri_sb = wts.tile([P, P], f16, name="tri_sb")
            idn_sb = wts.tile([P, P], f32, name="idn_sb")
            if has_qk_bias:
                xrow = wts.tile([1, T], f16, name="xrow")
                wrow = wts.tile([1, W_COLS], f16, name="wrow")
            # qT[p][c]/kT[p][c]: [128 dims (2 heads x 64), 512 T]
            qT = [[qk_res.tile([P, QC], f16, name=f"qT{p}_{c}")
                   for c in range(NQ)] for p in range(PAIRS)]
            kT = [[qk_res.tile([P, QC], f16, name=f"kT{p}_{c}")
                   for c in range(NQ)] for p in range(PAIRS)]
            # v_sb[tt]: [128 keys, 8 heads, 65 (v | 1)]
            v_sb = [v_res.tile([P, 8, 65], f16, name=f"v{t}")
                    for t in range(T // P)]
            # yT[p][c]: [128 dims, 512 T] (normalized, transposed)
            yT = [[yt_res.tile([P, QC], f16, name=f"yT{p}_{c}")
                   for c in range(NQ)] for p in range(PAIRS)]

            # ---------------- DMA loads ----------------
            # x chunk tiles; chunk 0 split in 4 pieces for early start
            xt = [None] * NQ
            xt[0] = xs_pool.tile([P, KT, QC], f16, name="xt", tag="xt")

            def load_x_piece(i):
                nc.sync.dma_start(
                    out=xt[0][:, 2 * i:2 * i + 2, :],
                    in_=xT[ts(i, 2 * P), ts(0, QC)].rearrange(
                        "(j p) q -> p j q", p=P))

            def load_wa(i, j):
                nc.sync.dma_start(
                    out=wa_sb[i][:, j, :],
                    in_=wa[ts(2 * i + j, P), :])

            # chunk-0 weight loads arrive in column-waves matching the
            # k-outer passes: wave 1 = qk cols 0:512 (pass A), wave 2 =
            # cols 512:1024 (pass B), wave 3 = v cols (pass C)
            def load_wa_cols(k, cc):
                nc.sync.dma_start(
                    out=wa_sb[k // 2][:, k % 2, ts(cc, QC)],
                    in_=wa[ts(k, P), ts(cc, QC)])

            load_wa_cols(0, 0)
            for i in range(4):
                load_x_piece(i)
                if 2 * i + 1 < KT:
                    load_wa_cols(2 * i + 1, 0)
                if 2 * i + 2 < KT:
                    load_wa_cols(2 * i + 2, 0)
            def load_x_chunk(n):
                xt[n] = xs_pool.tile([P, KT, QC], f16, name="xt", tag="xt")
                nc.sync.dma_start(
                    out=xt[n],
                    in_=xT[0:D_MODEL, ts(n, QC)].rearrange(
                        "(k p) q -> p k q", p=P))

            for k in range(KT):
                load_wa_cols(k, 1)
            for k in range(KT):
                load_wa_cols(k, 2)
            nc.sync.dma_start(out=tri_sb, in_=tri[:])
            nc.sync.dma_start(out=idn_sb, in_=idn[:])
            load_x_chunk(1)
            for j in range(PAIRS):
                nc.sync.dma_start(out=wp_sb[j], in_=wp[ts(j, P), :])
            if has_qk_bias:
                nc.sync.dma_start(out=xrow, in_=xT[D_MODEL:D_MODEL + 1, :])
                nc.sync.dma_start(out=wrow, in_=wa[D_MODEL:D_MODEL + 1, :])

            # ---------------- qkv projection helpers ----------------
            # w block col ranges: block 2p -> q pair p, 2p+1 -> k pair p,
            # block 8+j -> v (cols 1024 + j*128)
            def qk_copy_dest(blk, n):
                p, kind = divmod(blk, 2)
                return (qT if kind == 0 else kT)[p][n]

            def emit_qk_block(n, blk, ps):
                """Accumulate w-block x x-chunk into ps and copy to SBUF."""
                for k in range(KT):
                    nc.tensor.matmul(
                        ps[:], wa_sb[k // 2][:, k % 2, ts(blk, P)],
                        xt[n][:, k, :],
                        start=(k == 0),
                        stop=(k == KT - 1) and not has_qk_bias)
                if has_qk_bias:
                    nc.tensor.matmul(
                        ps[:], wrow[:, ts(blk, P)], xrow[:, ts(n, QC)],
                        start=False, stop=True)
                nc.vector.tensor_copy(out=qk_copy_dest(blk, n), in_=ps[:])

            def emit_v_block(n, tsub, ps):
                tt = 4 * n + tsub
                for k in range(KT):
                    nc.tensor.matmul(
                        ps[:], xt[n][:, k, ts(tsub, P)],
                        wa_sb[k // 2][:, k % 2, 1024:1536],
                        start=(k == 0),
                        stop=(k == KT - 1) and not has_qk_bias)
                if has_qk_bias:
                    nc.tensor.matmul(
                        ps[:], xrow[:, n * QC + tsub * P:
                                    n * QC + (tsub + 1) * P],
                        wrow[:, 1024:1536], start=False, stop=True)
                nc.vector.memset(v_sb[tt][:, :, 64:65], 1.0)
                nc.vector.tensor_copy(
                    out=v_sb[tt][:, :, 0:64],
                    in_=ps.rearrange("p (h c) -> p h c", c=64))

            def qkv_chunk_thunks(n):
                """Filler thunks for chunk n (needs xt[n] loaded)."""
                thunks = []
                for blk in range(8):
                    def t(blk=blk):
                        ps = lin_pool.tile([P, QC], f32, name="lps",
                                           tag="lps")
                        emit_qk_block(n, blk, ps)
                    thunks.append((t, 8 * QC * _PE_PER_ROW))
                for tsub in range(4):
                    def t(tsub=tsub):
                        ps = lin_pool.tile([P, QC], f32, name="lps",
                                           tag="lps")
                        emit_v_block(n, tsub, ps)
                    thunks.append((t, 8 * QC * _PE_PER_ROW))
                return thunks

            # ---------------- chunk 0: k-outer for early start ----------
            # 3 passes of 4 accumulators (2 lin tiles + 2 halves of an st
            # tile) so compute streams while wa/x DMAs land.
            for pass_blocks in ([0, 1, 2, 3], [4, 5, 6, 7],
                                [(0, 0), (0, 1), (0, 2), (0, 3)]):
                l0 = lin_pool.tile([P, QC], f32, name="lps", tag="lps")
                l1 = lin_pool.tile([P, QC], f32, name="lps", tag="lps")
                s0 = st_pool.tile([P, 2, QC], f32, name="st", tag="st")
                accs = [l0, l1, s0[:, 0, :], s0[:, 1, :]]
                is_v = isinstance(pass_blocks[0], tuple)
                for k in range(KT):
                    for a, blk in zip(accs, pass_blocks):
                        if is_v:
                            _, tsub = blk
                            nc.tensor.matmul(
                                a[:], xt[0][:, k, ts(tsub, P)],
                                wa_sb[k // 2][:, k % 2, 1024:1536],
                                start=(k == 0),
                                stop=(k == KT - 1) and not has_qk_bias)
                        else:
                            nc.tensor.matmul(
                                a[:], wa_sb[k // 2][:, k % 2, ts(blk, P)],
                                xt[0][:, k, :],
                                start=(k == 0),
                                stop=(k == KT - 1) and not has_qk_bias)
                for a, blk in zip(accs, pass_blocks):
                    if is_v:
                        _, tsub = blk
                        if has_qk_bias:
                            nc.tensor.matmul(
                                a[:], xrow[:, ts(tsub, P)],
                                wrow[:, 1024:1536], start=False, stop=True)
                        nc.vector.memset(v_sb[tsub][:, :, 64:65], 1.0)
                        nc.vector.tensor_copy(
                            out=v_sb[tsub][:, :, 0:64],
                            in_=a.rearrange("p (h c) -> p h c", c=64))
                    else:
                        if has_qk_bias:
                            nc.tensor.matmul(
                                a[:], wrow[:, ts(blk, P)], xrow[:, ts(0, QC)],
                                start=False, stop=True)
                        nc.vector.tensor_copy(out=qk_copy_dest(blk, 0),
                                              in_=a[:])

            # ---------------- attention ----------------
            def attn_chunk(c, pair_list, fillers, debt):
                """Attention for q-chunk c over pairs in pair_list.

                Software-pipelined: PV for (p, h) is emitted two h-slots
                after its QK groups, so the following pair's QK matmuls
                cover the exp latency. fillers: deque of (thunk, pe_ns);
                popped when the act-debt (exp time not covered by
                attention PE work) exceeds one thunk's worth.
                """
                # groups: (kt_a, kt_b, so_a, so_b)
                groups = [(2 * g, 2 * g + 1, 0, 0) for g in range(2 * c)]
                groups.append((4 * c, 4 * c + 1, 0, P))
                groups.append((4 * c + 2, 4 * c + 3, 2 * P, 3 * P))
                last_p = pair_list[-1]

                def pops():
                    while fillers and debt[0] > fillers[0][1]:
                        t, tns = fillers.popleft()
                        t()
                        debt[0] -= tns

                ysub_map = {}

                def emit_qk(p, h):
                    hb = h * 64
                    pieces = []
                    for ka, kb, soa, sob in groups:
                        st = st_pool.tile([P, 2, QC], f32, name="st",
                                          tag="st")
                        ex = ex_pool.tile([P, 2, QC], f16, name="ex",
                                          tag="ex")
                        for j, (kt, so) in enumerate(((ka, soa),
                                                      (kb, sob))):
                            nc.tensor.matmul(
                                st[:, j, so:QC],
                                kT[p][kt // 4][hb:hb + 64, ts(kt % 4, P)],
                                qT[p][c][hb:hb + 64, so:QC],
                                start=True, stop=True)
                            pieces.append((kt, so, ex, j))
                        # exp: one op per group over [min_so:512] of both
                        # pieces; the [soa:sob) part of piece b is unwritten
                        # PSUM whose exp is never read by PV
                        mso = min(soa, sob)
                        nc.scalar.activation(ex[:, :, mso:QC],
                                             st[:, :, mso:QC], EXPF,
                                             scale=0.125)
                        act_ns = (2 * (QC - mso) * _ACT_PER_ELEM
                                  + _ACT_PER_OP)
                        # mask the 128-wide diagonal transition blocks
                        pe_ns = 0.0
                        for j, (kt, so) in enumerate(((ka, soa),
                                                      (kb, sob))):
                            if kt >= 4 * c:
                                nc.vector.tensor_mul(
                                    ex[:, j, so:so + P],
                                    ex[:, j, so:so + P],
                                    tri_sb[:])
                            pe_ns += (QC - so) * _PE_PER_ROW
                        debt[0] += act_ns - pe_ns
                        pops()
                    return pieces

                def emit_pv(p, h, pieces):
                    lh = 2 * p + h
                    dst = ysub_map[p]
                    finalize = (c == 3 and p == last_p and h == 1)
                    y_ps = y_pool.tile([P, 4, P], f32, name="yps",
                                       tag="yps")
                    if finalize:
                        tp = st_pool.tile([P, 2, QC], f32, name="st",
                                          tag="st")

                    def fin_transpose(qs):
                        nc.tensor.transpose(tp[:, qs % 2, 0:P],
                                            dst[qs][:], idn_sb[:])
                        nc.vector.tensor_copy(
                            out=yT[p][c][:, ts(qs, P)],
                            in_=tp[:, qs % 2, 0:P])

                    for qs in range(4):
                        last_kt = 4 * c + qs
                        for kt, so, ex, j in pieces:
                            if kt > last_kt:
                                continue
                            jj = kt - 4 * c
                            if jj >= 0 and qs < jj:
                                continue
                            nc.tensor.matmul(
                                y_ps[:, qs, 0:65],
                                ex[:, j, ts(qs, P)],
                                v_sb[kt][:, lh, :],
                                start=(kt == 0),
                                stop=(kt == last_kt))
                            debt[0] -= 65 * _PE_PER_ROW
                            pops()
                        if finalize:
                            # per-qs normalize then PE transpose of the
                            # previous qs, pipelining the tail chain
                            rq = r_pool.tile([P, 4], f32, name="rcp",
                                             tag="rcp")
                            nc.vector.reciprocal(
                                out=rq[:, 0:1], in_=y_ps[:, qs, 64:65])
                            nc.vector.tensor_scalar_mul(
                                out=dst[qs][:, h, :],
                                in0=y_ps[:, qs, 0:64],
                                scalar1=rq[:, 0:1])
                            if qs >= 1:
                                fin_transpose(qs - 1)
                    if finalize:
                        fin_transpose(3)
                        return
                    # normalize: recip of denominators, scale 64 v-dims
                    r = r_pool.tile([P, 4], f32, name="rcp", tag="rcp")
                    nc.vector.reciprocal(out=r[:], in_=y_ps[:, :, 64:65])
                    for qs in range(4):
                        nc.vector.tensor_scalar_mul(
                            out=dst[qs][:, h, :],
                            in0=y_ps[:, qs, 0:64],
                            scalar1=r[:, qs:qs + 1])
                    if h == 1:
                        # transpose y [128 q, 128 dims] -> yT [dims, q]
                        for qs in range(4):
                            nc.sync.dma_start(
                                out=yT[p][c][:, ts(qs, P)],
                                in_=dst[qs][:], transpose=True)

                pending = collections.deque()
                for p in pair_list:
                    if c == 3 and p == last_p:
                        ysub_map[p] = [
                            ysb_pool.tile([P, 2, 64], f32, name="ysbf",
                                          tag="ysbf") for _ in range(4)]
                    else:
                        ysub_map[p] = [
                            ysb_pool.tile([P, 2, 64], f16, name="ysb",
                                          tag="ysb") for _ in range(4)]
                    for h in (0, 1):
                        pending.append((p, h, emit_qk(p, h)))
                        if len(pending) == 4:
                            emit_pv(*pending.popleft())
                while pending:
                    emit_pv(*pending.popleft())

            def proj_thunks(c, split_dma=False):
                """Partial out-proj for T-tiles of chunk c (all pairs)."""
                thunks = []
                osb = {}

                def mk(tt, half):
                    def t():
                        if half == 0 and not split_dma:
                            osb[tt] = o_pool.tile([P, 1024], f16, name="osb",
                                                  tag="osb")
                        ps = lin_pool.tile([P, QC], f32, name="lps",
                                           tag="lps")
                        for p in range(PAIRS):
                            nc.tensor.matmul(
                                ps[:], yT[p][tt // 4][:, ts(tt % 4, P)],
                                wp_sb[p][:, ts(half, QC)],
                                start=(p == 0), stop=(p == PAIRS - 1))
                        if split_dma:
                            ob = o_pool.tile([P, 1024], f16, name="osb",
                                             tag="osb")
                            nc.vector.tensor_copy(out=ob[:, 0:QC], in_=ps[:])
                            nc.sync.dma_start(
                                out=out[ts(tt, P), ts(half, QC)],
                                in_=ob[:, 0:QC])
                        else:
                            nc.vector.tensor_copy(
                                out=osb[tt][:, ts(half, QC)], in_=ps[:])
                            if half == 1:
                                nc.sync.dma_start(out=out[ts(tt, P), :],
                                                  in_=osb[tt][:])
                    return t

                for tt in range(4 * c, 4 * c + 4):
                    for half in (0, 1):
                        thunks.append((mk(tt, half),
                                       PAIRS * QC * _PE_PER_ROW))
                return thunks

            # ---------------- main schedule ----------------
            debt = [0.0]
            fillers = collections.deque(qkv_chunk_thunks(1))
            attn_chunk(0, range(PAIRS), fillers, debt)
            while fillers:
                fillers.popleft()[0]()

            load_x_chunk(2)
            fillers = collections.deque(qkv_chunk_thunks(2))
            attn_chunk(1, range(PAIRS), fillers, debt)
            while fillers:
                fillers.popleft()[0]()

            load_x_chunk(3)
            fillers = collections.deque(qkv_chunk_thunks(3))
            attn_chunk(2, range(PAIRS), fillers, debt)
            while fillers:
                fillers.popleft()[0]()

            # proj(3): keep two groups' pair-0..2 matmuls in flight ahead
            # of the pair-3 close so the last transposes are hidden
            open_g = collections.deque()

            def open_group(tt, half):
                ps = lin_pool.tile([P, QC], f32, name="lps", tag="lps")
                for p in range(PAIRS - 1):
                    nc.tensor.matmul(
                        ps[:], yT[p][tt // 4][:, ts(tt % 4, P)],
                        wp_sb[p][:, ts(half, QC)],
                        start=(p == 0), stop=False)
                open_g.append((tt, half, ps))

            osb3 = {}

            def close_group():
                tt, half, ps = open_g.popleft()
                nc.tensor.matmul(
                    ps[:], yT[PAIRS - 1][tt // 4][:, ts(tt % 4, P)],
                    wp_sb[PAIRS - 1][:, ts(half, QC)],
                    start=False, stop=True)
                if half == 0:
                    osb3[tt] = o_pool.tile([P, 1024], f16, name="osb",
                                           tag="osb")
                nc.vector.tensor_copy(out=osb3[tt][:, ts(half, QC)],
                                      in_=ps[:])
                if tt == 15:
                    # final tile: DMA each half as soon as it is copied
                    nc.sync.dma_start(out=out[ts(tt, P), ts(half, QC)],
                                      in_=osb3[tt][:, ts(half, QC)])
                elif half == 1:
                    nc.sync.dma_start(out=out[ts(tt, P), :],
                                      in_=osb3[tt][:])

            fillers = collections.deque(
                proj_thunks(0) + proj_thunks(1) + proj_thunks(2))
            g3 = [(tt, half) for tt in range(12, 16) for half in (0, 1)]
            for tt, half in g3[:2]:
                fillers.append(
                    (lambda tt=tt, half=half: open_group(tt, half),
                     (PAIRS - 1) * QC * _PE_PER_ROW))
            attn_chunk(3, range(PAIRS), fillers, debt)
            while fillers:
                fillers.popleft()[0]()

            for tt, half in g3[2:]:
                close_group()
                open_group(tt, half)
            while open_g:
                close_group()

    nc.compile()
    return nc


def _make_runner(nc):
    """Reusable 8-core SPMD runner (jit built once)."""
    import jax
    from jax.sharding import Mesh, PartitionSpec
    from jax.experimental.shard_map import shard_map
    from concourse import bass2jax
    import concourse.mybir as mybir

    bass2jax.install_neuronx_cc_hook()
    partition_name = (nc.partition_id_tensor.name
                      if nc.partition_id_tensor else None)
    in_names, out_names, out_avals, zero_outs = [], [], [], []
    for alloc in nc.m.functions[0].allocations:
        if not isinstance(alloc, mybir.MemoryLocationSet):
            continue
        name = alloc.memorylocations[0].name
        if alloc.kind == "ExternalInput":
            if name != partition_name:
                in_names.append(name)
        elif alloc.kind == "ExternalOutput":
            shape = tuple(alloc.tensor_shape)
            dtype = mybir.dt.np(alloc.dtype)
            out_names.append(name)
            out_avals.append(jax.core.ShapedArray(shape, dtype))
            zero_outs.append(np.zeros(shape, dtype))
    n_params = len(in_names)
    n_outs = len(out_avals)
    all_in = list(in_names) + list(out_names)
    if partition_name is not None:
        all_in.append(partition_name)

    def _body(*args):
        operands = list(args)
        if partition_name is not None:
            operands.append(bass2jax.partition_id_tensor())
        outs = bass2jax._bass_exec_p.bind(
            *operands,
            out_avals=tuple(out_avals),
            in_names=tuple(all_in),
            out_names=tuple(out_names),
            lowering_input_output_aliases=(),
            sim_require_finite=True,
            sim_require_nnan=True,
            nc=nc,
        )
        return tuple(outs)

    devices = jax.devices()[:N_CORES]
    mesh = Mesh(np.asarray(devices), ("core",))
    in_specs = (PartitionSpec("core"),) * (n_params + n_outs)
    out_specs = (PartitionSpec("core"),) * n_outs
    donate = tuple(range(n_params, n_params + n_outs))
    sharded = jax.jit(
        shard_map(_body, mesh=mesh, in_specs=in_specs, out_specs=out_specs,
                  check_rep=False),
        donate_argnums=donate, keep_unused=True)

    def run(in_maps):
        per_core = [[np.asarray(m[k]) for k in in_names] for m in in_maps]
        concat_in = [
            np.concatenate([per_core[c][i] for c in range(N_CORES)], axis=0)
            for i in range(n_params)]
        concat_zeros = [
            np.zeros((N_CORES * z.shape[0], *z.shape[1:]), z.dtype)
            for z in zero_outs]
        outs = sharded(*concat_in, *concat_zeros)
        jax.block_until_ready(outs)
        return [
            {name: np.asarray(outs[i]).reshape(N_CORES, *out_avals[i].shape)[c]
             for i, name in enumerate(out_names)}
            for c in range(N_CORES)]

    return run


def kernel(x, w_qkv, b_qkv, w_proj, b_proj):
    x = np.asarray(x, dtype=np.float32)
    w_qkv = np.asarray(w_qkv, dtype=np.float32)
    b_qkv = np.asarray(b_qkv, dtype=np.float32)
    w_proj = np.asarray(w_proj, dtype=np.float32)
    b_proj = np.asarray(b_proj, dtype=np.float32)

    w_q, w_k, w_v = w_qkv[0:1024], w_qkv[1024:2048], w_qkv[2048:3072]
    b_q, b_k, b_v = b_qkv[0:1024], b_qkv[1024:2048], b_qkv[2048:3072]
    has_qk_bias = bool(np.any(b_q) or np.any(b_k))

    key = ("runner", has_qk_bias)
    if key not in _RUNNER_CACHE:
        nc = _build(has_qk_bias)
        _RUNNER_CACHE[key] = _make_runner(nc)
    run = _RUNNER_CACHE[key]

    # causal transition-block mask: tri[k, i] = 1.0 iff k <= i
    kk = np.arange(P)
    tri = (kk[:, None] <= kk[None, :]).astype(np.float16)
    idn = np.eye(P, dtype=np.float32)

    in_maps = []
    for core in range(N_CORES):
        b, g = divmod(core, 2)
        xT_c = x[b].T.astype(np.float16)  # [1024, 2048]
        if has_qk_bias:
            xT_c = np.concatenate(
                [xT_c, np.ones((1, T), np.float16)], axis=0)
        KD = D_MODEL + (1 if has_qk_bias else 0)
        wa_c = np.empty((KD, W_COLS), np.float32)
        wp_c = np.empty((512, 1024), np.float32)
        for p in range(PAIRS):
            hA = 8 * g + 2 * p
            hB = hA + 1
            cols = p * 256
            wa_c[:D_MODEL, cols + 0:cols + 64] = w_q[hA * 64:(hA + 1) * 64].T
            wa_c[:D_MODEL, cols + 64:cols + 128] = w_q[hB * 64:(hB + 1) * 64].T
            wa_c[:D_MODEL, cols + 128:cols + 192] = w_k[hA * 64:(hA + 1) * 64].T
            wa_c[:D_MODEL, cols + 192:cols + 256] = w_k[hB * 64:(hB + 1) * 64].T
            if has_qk_bias:
                wa_c[D_MODEL, cols + 0:cols + 64] = b_q[hA * 64:(hA + 1) * 64]
                wa_c[D_MODEL, cols + 64:cols + 128] = b_q[hB * 64:(hB + 1) * 64]
                wa_c[D_MODEL, cols + 128:cols + 192] = b_k[hA * 64:(hA + 1) * 64]
                wa_c[D_MODEL, cols + 192:cols + 256] = b_k[hB * 64:(hB + 1) * 64]
            # wp rows pair-major: [hA dims 0..63 | hB dims 64..127]
            wp_c[p * 128:p * 128 + 64, :] = w_proj.T[hA * 64:(hA + 1) * 64, :]
            wp_c[p * 128 + 64:p * 128 + 128, :] = \
                w_proj.T[hB * 64:(hB + 1) * 64, :]
        # v columns, head-major for the group
        for lh in range(8):
            head = 8 * g + lh
            wa_c[:D_MODEL, 1024 + lh * 64:1024 + (lh + 1) * 64] = \
                w_v[head * 64:(head + 1) * 64].T
            if has_qk_bias:
                wa_c[D_MODEL, 1024 + lh * 64:1024 + (lh + 1) * 64] = \
                    b_v[head * 64:(head + 1) * 64]
        in_maps.append({
            "xT": xT_c,
            "wa": wa_c.astype(np.float16),
            "wp": wp_c.astype(np.float16),
            "tri": tri,
            "idn": idn,
        })

    results = run(in_maps)

    # partial-sum unshard: the two head-group cores of each batch each
    # produced out_partial[T, 1024]; add them.
    out = np.empty((B, T, D_MODEL), dtype=np.float32)
    for b in range(B):
        out[b] = (results[2 * b]["out"].astype(np.float32)
                  + results[2 * b + 1]["out"].astype(np.float32))

    # exact host-side bias folds (v-bias rides softmax row-sums == 1 and is
    # on-device in the qk-bias build; proj bias is additive)
    if np.any(b_v) and not has_qk_bias:
        out += (b_v @ w_proj.T)[None, None, :]
    if np.any(b_proj):
        out += b_proj[None, None, :]
    return out


# revision 8
# speedup vs baseline: 1.0818x; 1.0358x over previous
"""Causal self-attention (B=4, T=2048, C=1024, H=16) on 8 TRN2 NeuronCores.

Sharding: core = 2*b + g (b = batch 0..3, g = head-group 0..1). Each core
computes qkv + attention for its batch and its 8 heads, then a PARTIAL
output projection over its own 512 y-dims for ALL 1024 output columns.
The host adds the two partials per batch (partial-sum unsharding) - no
device collectives at all.

All matmuls run in fp16 (1 PE cycle/row, no min-width constraint),
accumulation in fp32 PSUM. Attention PV uses the cheap orientation
out[q,65] = ex^T @ [V | 1] (65 rows per (k-tile, q-subtile) instead of
512), with the softmax denominator from the ones column; y is normalized
with a per-partition scalar multiply and transposed back to [dims, T]
with the DMA XBAR transpose (off the PE).

The attention inner loop is Activation-engine paced (exp); qkv-proj and
out-proj matmul "filler" work is interleaved between score/PV groups via
a debt counter so the PE never idles waiting for exp.
"""
import collections
import numpy as np

D_MODEL = 1024
N_HEAD = 16
D_HEAD = 64
B = 4
T = 2048
N_CORES = 8
P = 128
PAIRS = 4          # head pairs per core
NQ = 4             # q-chunks of 512
QC = 512           # q chunk width
KT = D_MODEL // P  # 8 contraction tiles for the qkv projection
W_COLS = 1536      # 1024 q/k cols + 512 v cols per core

_RUNNER_CACHE = {}

# cost-model-ish estimates (ns) for the act-debt interleaver
_ACT_PER_ELEM = 0.833
_ACT_PER_OP = 185.0
_PE_PER_ROW = 0.4167


def _build(has_qk_bias: bool, _nphases: int = 5):
    from concourse import bacc
    import concourse.mybir as mybir
    from concourse.tile import TileContext
    from concourse.bass import ts

    f32 = mybir.dt.float32
    f16 = mybir.dt.float16
    KD = D_MODEL + (1 if has_qk_bias else 0)

    nc = bacc.Bacc("TRN2", target_bir_lowering=False, debug=False,
                   num_devices=N_CORES)
    xT = nc.dram_tensor("xT", [KD, T], f16, kind="ExternalInput")
    wa = nc.dram_tensor("wa", [KD, W_COLS], f16, kind="ExternalInput")
    wp = nc.dram_tensor("wp", [512, 1024], f16, kind="ExternalInput")
    tri = nc.dram_tensor("tri", [P, P], f16, kind="ExternalInput")
    idn = nc.dram_tensor("idn", [P, P], f32, kind="ExternalInput")
    out = nc.dram_tensor("out", [T, 1024], f16, kind="ExternalOutput")

    EXPF = mybir.ActivationFunctionType.Exp

    with TileContext(nc) as tc:
        with (
            tc.tile_pool(name="wts", bufs=1) as wts,
            tc.tile_pool(name="qk_res", bufs=1) as qk_res,
            tc.tile_pool(name="v_res", bufs=1) as v_res,
            tc.tile_pool(name="yt_res", bufs=1) as yt_res,
            tc.tile_pool(name="xs", bufs=2) as xs_pool,
            tc.tile_pool(name="exp", bufs=40) as ex_pool,
            tc.tile_pool(name="ysb", bufs=8) as ysb_pool,
            tc.tile_pool(name="rcp", bufs=4) as r_pool,
            tc.tile_pool(name="osb", bufs=4) as o_pool,
            tc.tile_pool(name="lin", bufs=2, space="PSUM") as lin_pool,
            tc.tile_pool(name="st", bufs=2, space="PSUM") as st_pool,
            tc.tile_pool(name="yps", bufs=2, space="PSUM") as y_pool,
        ):
            # ---------------- persistent tiles ----------------
            # wa_sb[i] holds contraction k-tiles 2i, 2i+1: [128, 2, 1536]
            wa_sb = [wts.tile([P, 2, W_COLS], f16, name=f"wa{i}")
                     for i in range(4)]
            wp_sb = [wts.tile([P, 1024], f16, name=f"wp{j}")
                     for j in range(PAIRS)]
            tri_sb = wts.tile([P, P], f16, name="tri_sb")
            idn_sb = wts.tile([P, P], f32, name="idn_sb")
            if has_qk_bias:
                xrow = wts.tile([1, T], f16, name="xrow")
                wrow = wts.tile([1, W_COLS], f16, name="wrow")
            # qT[p][c]/kT[p][c]: [128 dims (2 heads x 64), 512 T]
            qT = [[qk_res.tile([P, QC], f16, name=f"qT{p}_{c}")
                   for c in range(NQ)] for p in range(PAIRS)]
            kT = [[qk_res.tile([P, QC], f16, name=f"kT{p}_{c}")
                   for c in range(NQ)] for p in range(PAIRS)]
            # v_sb[tt]: [128 keys, 8 heads, 65 (v | 1)]
            v_sb = [v_res.tile([P, 8, 65], f16, name=f"v{t}")
                    for t in range(T // P)]
            # yT[p][c]: [128 dims, 512 T] (normalized, transposed)
            yT = [[yt_res.tile([P, QC], f16, name=f"yT{p}_{c}")
                   for c in range(NQ)] for p in range(PAIRS)]

            # ---------------- DMA loads ----------------
            # x chunk tiles; chunk 0 split in 4 pieces for early start
            xt = [None] * NQ
            xt[0] = xs_pool.tile([P, KT, QC], f16, name="xt", tag="xt")

            def load_x_piece(i):
                nc.sync.dma_start(
                    out=xt[0][:, 2 * i:2 * i + 2, :],
                    in_=xT[ts(i, 2 * P), ts(0, QC)].rearrange(
                        "(j p) q -> p j q", p=P))

            def load_wa(i, j):
                nc.sync.dma_start(
                    out=wa_sb[i][:, j, :],
                    in_=wa[ts(2 * i + j, P), :])

            # chunk-0 weight loads arrive in column-waves matching the
            # k-outer passes: wave 1 = qk cols 0:512 (pass A), wave 2 =
            # cols 512:1024 (pass B), wave 3 = v cols (pass C)
            def load_wa_cols(k, cc):
                nc.sync.dma_start(
                    out=wa_sb[k // 2][:, k % 2, ts(cc, QC)],
                    in_=wa[ts(k, P), ts(cc, QC)])

            load_wa_cols(0, 0)
            for i in range(4):
                load_x_piece(i)
                if 2 * i + 1 < KT:
                    load_wa_cols(2 * i + 1, 0)
                if 2 * i + 2 < KT:
                    load_wa_cols(2 * i + 2, 0)
            def load_x_chunk(n):
                xt[n] = xs_pool.tile([P, KT, QC], f16, name="xt", tag="xt")
                nc.sync.dma_start(
                    out=xt[n],
                    in_=xT[0:D_MODEL, ts(n, QC)].rearrange(
                        "(k p) q -> p k q", p=P))

            for k in range(KT):
                load_wa_cols(k, 1)
            for k in range(KT):
                load_wa_cols(k, 2)
            nc.sync.dma_start(out=tri_sb, in_=tri[:])
            nc.sync.dma_start(out=idn_sb, in_=idn[:])
            load_x_chunk(1)
            for j in range(PAIRS):
                nc.sync.dma_start(out=wp_sb[j], in_=wp[ts(j, P), :])
            if has_qk_bias:
                nc.sync.dma_start(out=xrow, in_=xT[D_MODEL:D_MODEL + 1, :])
                nc.sync.dma_start(out=wrow, in_=wa[D_MODEL:D_MODEL + 1, :])

            # ---------------- qkv projection helpers ----------------
            # w block col ranges: block 2p -> q pair p, 2p+1 -> k pair p,
            # block 8+j -> v (cols 1024 + j*128)
            def qk_copy_dest(blk, n):
                p, kind = divmod(blk, 2)
                return (qT if kind == 0 else kT)[p][n]

            def emit_qk_block(n, blk, ps):
                """Accumulate w-block x x-chunk into ps and copy to SBUF."""
                for k in range(KT):
                    nc.tensor.matmul(
                        ps[:], wa_sb[k // 2][:, k % 2, ts(blk, P)],
                        xt[n][:, k, :],
                        start=(k == 0),
                        stop=(k == KT - 1) and not has_qk_bias)
                if has_qk_bias:
                    nc.tensor.matmul(
                        ps[:], wrow[:, ts(blk, P)], xrow[:, ts(n, QC)],
                        start=False, stop=True)
                nc.vector.tensor_copy(out=qk_copy_dest(blk, n), in_=ps[:])

            def emit_v_block(n, tsub, ps):
                tt = 4 * n + tsub
                for k in range(KT):
                    nc.tensor.matmul(
                        ps[:], xt[n][:, k, ts(tsub, P)],
                        wa_sb[k // 2][:, k % 2, 1024:1536],
                        start=(k == 0),
                        stop=(k == KT - 1) and not has_qk_bias)
                if has_qk_bias:
                    nc.tensor.matmul(
                        ps[:], xrow[:, n * QC + tsub * P:
                                    n * QC + (tsub + 1) * P],
                        wrow[:, 1024:1536], start=False, stop=True)
                nc.vector.memset(v_sb[tt][:, :, 64:65], 1.0)
                nc.vector.tensor_copy(
                    out=v_sb[tt][:, :, 0:64],
                    in_=ps.rearrange("p (h c) -> p h c", c=64))

            def qkv_chunk_thunks(n):
                """Filler thunks for chunk n (needs xt[n] loaded)."""
                thunks = []
                for blk in range(8):
                    def t(blk=blk):
                        ps = lin_pool.tile([P, QC], f32, name="lps",
                                           tag="lps")
                        emit_qk_block(n, blk, ps)
                    thunks.append((t, 8 * QC * _PE_PER_ROW))
                for tsub in range(4):
                    def t(tsub=tsub):
                        ps = lin_pool.tile([P, QC], f32, name="lps",
                                           tag="lps")
                        emit_v_block(n, tsub, ps)
                    thunks.append((t, 8 * QC * _PE_PER_ROW))
                return thunks

            # ---------------- chunk 0: k-outer for early start ----------
            # 3 passes of 4 accumulators (2 lin tiles + 2 halves of an st
            # tile) so compute streams while wa/x DMAs land.
            for pass_blocks in ([0, 1, 2, 3], [4, 5, 6, 7],
                                [(0, 0), (0, 1), (0, 2), (0, 3)]):
                l0 = lin_pool.tile([P, QC], f32, name="lps", tag="lps")
                l1 = lin_pool.tile([P, QC], f32, name="lps", tag="lps")
                s0 = st_pool.tile([P, 2, QC], f32, name="st", tag="st")
                accs = [l0, l1, s0[:, 0, :], s0[:, 1, :]]
                is_v = isinstance(pass_blocks[0], tuple)
                for k in range(KT):
                    for a, blk in zip(accs, pass_blocks):
                        if is_v:
                            _, tsub = blk
                            nc.tensor.matmul(
                                a[:], xt[0][:, k, ts(tsub, P)],
                                wa_sb[k // 2][:, k % 2, 1024:1536],
                                start=(k == 0),
                                stop=(k == KT - 1) and not has_qk_bias)
                        else:
                            nc.tensor.matmul(
                                a[:], wa_sb[k // 2][:, k % 2, ts(blk, P)],
                                xt[0][:, k, :],
                                start=(k == 0),
                                stop=(k == KT - 1) and not has_qk_bias)
                for a, blk in zip(accs, pass_blocks):
                    if is_v:
                        _, tsub = blk
                        if has_qk_bias:
                            nc.tensor.matmul(
                                a[:], xrow[:, ts(tsub, P)],
                                wrow[:, 1024:1536], start=False, stop=True)
                        nc.vector.memset(v_sb[tsub][:, :, 64:65], 1.0)
                        nc.vector.tensor_copy(
                            out=v_sb[tsub][:, :, 0:64],
                            in_=a.rearrange("p (h c) -> p h c", c=64))
                    else:
                        if has_qk_bias:
                            nc.tensor.matmul(
                                a[:], wrow[:, ts(blk, P)], xrow[:, ts(0, QC)],
                                start=False, stop=True)
                        nc.vector.tensor_copy(out=qk_copy_dest(blk, 0),
                                              in_=a[:])

            # ---------------- attention ----------------
            def attn_chunk(c, pair_list, fillers, debt):
                """Attention for q-chunk c over pairs in pair_list.

                Software-pipelined: PV for (p, h) is emitted two h-slots
                after its QK groups, so the following pair's QK matmuls
                cover the exp latency. fillers: deque of (thunk, pe_ns);
                popped when the act-debt (exp time not covered by
                attention PE work) exceeds one thunk's worth.
                """
                # groups: (kt_a, kt_b, so_a, so_b)
                groups = [(2 * g, 2 * g + 1, 0, 0) for g in range(2 * c)]
                groups.append((4 * c, 4 * c + 1, 0, P))
                groups.append((4 * c + 2, 4 * c + 3, 2 * P, 3 * P))
                last_p = pair_list[-1]

                def pops():
                    while fillers and debt[0] > fillers[0][1]:
                        t, tns = fillers.popleft()
                        t()
                        debt[0] -= tns

                ysub_map = {}

                def emit_qk(p, h):
                    hb = h * 64
                    pieces = []
                    for ka, kb, soa, sob in groups:
                        st = st_pool.tile([P, 2, QC], f32, name="st",
                                          tag="st")
                        ex = ex_pool.tile([P, 2, QC], f16, name="ex",
                                          tag="ex")
                        # piece spec: (kt, so, bank j, store offset sho)
                        if sob == 3 * P:
                            # diagonal pair B: pack both pieces (widths
                            # 256 + 128) into bank 0 so one exact exp op
                            # covers them
                            spec = [(ka, soa, 0, 0), (kb, sob, 0, 2 * P)]
                            exp_ops = [((slice(None), 0,
                                         slice(0, 3 * P)), 3 * P)]
                        elif sob == P:
                            # diagonal pair A: merged exp over [0:512] (the
                            # [0:P) part of piece b is unwritten PSUM whose
                            # exp is never read by PV)
                            spec = [(ka, soa, 0, soa), (kb, sob, 1, sob)]
                            exp_ops = [((slice(None), slice(None),
                                         slice(0, QC)), 2 * QC)]
                        else:
                            spec = [(ka, soa, 0, soa), (kb, sob, 1, sob)]
                            exp_ops = [((slice(None), slice(None),
                                         slice(0, QC)), 2 * QC)]
                        for kt, so, j, sho in spec:
                            w = QC - so
                            nc.tensor.matmul(
                                st[:, j, sho:sho + w],
                                kT[p][kt // 4][hb:hb + 64, ts(kt % 4, P)],
                                qT[p][c][hb:hb + 64, so:QC],
                                start=True, stop=True)
                            pieces.append((kt, so, ex, j, sho))
                        act_ns = 0.0
                        for sl, elems in exp_ops:
                            nc.scalar.activation(ex[sl], st[sl], EXPF,
                                                 scale=0.125)
                            act_ns += elems * _ACT_PER_ELEM + _ACT_PER_OP
                        # mask the 128-wide diagonal transition blocks
                        pe_ns = 0.0
                        for kt, so, j, sho in spec:
                            if kt >= 4 * c:
                                nc.vector.tensor_mul(
                                    ex[:, j, sho:sho + P],
                                    ex[:, j, sho:sho + P],
                                    tri_sb[:])
                            pe_ns += (QC - so) * _PE_PER_ROW
                        debt[0] += act_ns - pe_ns
                        pops()
                    return pieces

                def emit_pv(p, h, pieces):
                    lh = 2 * p + h
                    dst = ysub_map[p]
                    finalize = (c == 3 and p == last_p and h == 1)
                    y_ps = y_pool.tile([P, 4, P], f32, name="yps",
                                       tag="yps")
                    if finalize:
                        tp = st_pool.tile([P, 2, QC], f32, name="st",
                                          tag="st")

                    def fin_transpose(qs):
                        nc.tensor.transpose(tp[:, qs % 2, 0:P],
                                            dst[qs][:], idn_sb[:])
                        nc.vector.tensor_copy(
                            out=yT[p][c][:, ts(qs, P)],
                            in_=tp[:, qs % 2, 0:P])

                    for qs in range(4):
                        last_kt = 4 * c + qs
                        for kt, so, ex, j, sho in pieces:
                            if kt > last_kt:
                                continue
                            jj = kt - 4 * c
                            if jj >= 0 and qs < jj:
                                continue
                            off = sho + qs * P - so
                            nc.tensor.matmul(
                                y_ps[:, qs, 0:65],
                                ex[:, j, off:off + P],
                                v_sb[kt][:, lh, :],
                                start=(kt == 0),
                                stop=(kt == last_kt))
                            debt[0] -= 65 * _PE_PER_ROW
                            pops()
                        if finalize:
                            # per-qs normalize then PE transpose of the
                            # previous qs, pipelining the tail chain
                            rq = r_pool.tile([P, 4], f32, name="rcp",
                                             tag="rcp")
                            nc.vector.reciprocal(
                                out=rq[:, 0:1], in_=y_ps[:, qs, 64:65])
                            nc.vector.tensor_scalar_mul(
                                out=dst[qs][:, h, :],
                                in0=y_ps[:, qs, 0:64],
                                scalar1=rq[:, 0:1])
                            if qs >= 1:
                                fin_transpose(qs - 1)
                    if finalize:
                        fin_transpose(3)
                        return
                    # normalize: recip of denominators, scale 64 v-dims
                    r = r_pool.tile([P, 4], f32, name="rcp", tag="rcp")
                    nc.vector.reciprocal(out=r[:], in_=y_ps[:, :, 64:65])
                    for qs in range(4):
                        nc.vector.tensor_scalar_mul(
                            out=dst[qs][:, h, :],
                            in0=y_ps[:, qs, 0:64],
                            scalar1=r[:, qs:qs + 1])
                    if h == 1:
                        # transpose y [128 q, 128 dims] -> yT [dims, q]
                        for qs in range(4):
                            nc.sync.dma_start(
                                out=yT[p][c][:, ts(qs, P)],
                                in_=dst[qs][:], transpose=True)

                pending = collections.deque()
                for p in pair_list:
                    if c == 3 and p == last_p:
                        ysub_map[p] = [
                            ysb_pool.tile([P, 2, 64], f32, name="ysbf",
                                          tag="ysbf") for _ in range(4)]
                    else:
                        ysub_map[p] = [
                            ysb_pool.tile([P, 2, 64], f16, name="ysb",
                                          tag="ysb") for _ in range(4)]
                    for h in (0, 1):
                        pending.append((p, h, emit_qk(p, h)))
                        if len(pending) == 4:
                            emit_pv(*pending.popleft())
                while pending:
                    emit_pv(*pending.popleft())

            def proj_thunks(c, split_dma=False):
                """Partial out-proj for T-tiles of chunk c (all pairs)."""
                thunks = []
                osb = {}

                def mk(tt, half):
                    def t():
                        if half == 0 and not split_dma:
                            osb[tt] = o_pool.tile([P, 1024], f16, name="osb",
                                                  tag="osb")
                        ps = lin_pool.tile([P, QC], f32, name="lps",
                                           tag="lps")
                        for p in range(PAIRS):
                            nc.tensor.matmul(
                                ps[:], yT[p][tt // 4][:, ts(tt % 4, P)],
                                wp_sb[p][:, ts(half, QC)],
                                start=(p == 0), stop=(p == PAIRS - 1))
                        if split_dma:
                            ob = o_pool.tile([P, 1024], f16, name="osb",
                                             tag="osb")
                            nc.vector.tensor_copy(out=ob[:, 0:QC], in_=ps[:])
                            nc.sync.dma_start(
                                out=out[ts(tt, P), ts(half, QC)],
                                in_=ob[:, 0:QC])
                        else:
                            nc.vector.tensor_copy(
                                out=osb[tt][:, ts(half, QC)], in_=ps[:])
                            if half == 1:
                                nc.sync.dma_start(out=out[ts(tt, P), :],
                                                  in_=osb[tt][:])
                    return t

                for tt in range(4 * c, 4 * c + 4):
                    for half in (0, 1):
                        thunks.append((mk(tt, half),
                                       PAIRS * QC * _PE_PER_ROW))
                return thunks

            # ---------------- main schedule ----------------
            debt = [0.0]
            fillers = collections.deque(qkv_chunk_thunks(1))
            attn_chunk(0, range(PAIRS), fillers, debt)
            while fillers:
                fillers.popleft()[0]()

            load_x_chunk(2)
            fillers = collections.deque(qkv_chunk_thunks(2))
            attn_chunk(1, range(PAIRS), fillers, debt)
            while fillers:
                fillers.popleft()[0]()

            load_x_chunk(3)
            fillers = collections.deque(qkv_chunk_thunks(3))
            attn_chunk(2, range(PAIRS), fillers, debt)
            while fillers:
                fillers.popleft()[0]()

            # proj(3): keep two groups' pair-0..2 matmuls in flight ahead
            # of the pair-3 close so the last transposes are hidden
            open_g = collections.deque()

            def open_group(tt, half):
                ps = lin_pool.tile([P, QC], f32, name="lps", tag="lps")
                for p in range(PAIRS - 1):
                    nc.tensor.matmul(
                        ps[:], yT[p][tt // 4][:, ts(tt % 4, P)],
                        wp_sb[p][:, ts(half, QC)],
                        start=(p == 0), stop=False)
                open_g.append((tt, half, ps))

            osb3 = {}

            def close_group():
                tt, half, ps = open_g.popleft()
                nc.tensor.matmul(
                    ps[:], yT[PAIRS - 1][tt // 4][:, ts(tt % 4, P)],
                    wp_sb[PAIRS - 1][:, ts(half, QC)],
                    start=False, stop=True)
                if half == 0:
                    osb3[tt] = o_pool.tile([P, 1024], f16, name="osb",
                                           tag="osb")
                nc.vector.tensor_copy(out=osb3[tt][:, ts(half, QC)],
                                      in_=ps[:])
                if tt == 15:
                    # final tile: DMA each half as soon as it is copied
                    nc.sync.dma_start(out=out[ts(tt, P), ts(half, QC)],
                                      in_=osb3[tt][:, ts(half, QC)])
                elif half == 1:
                    nc.sync.dma_start(out=out[ts(tt, P), :],
                                      in_=osb3[tt][:])

            fillers = collections.deque(
                proj_thunks(0) + proj_thunks(1) + proj_thunks(2))
            g3 = [(tt, half) for tt in range(12, 16) for half in (0, 1)]
            for tt, half in g3[:2]:
                fillers.append(
                    (lambda tt=tt, half=half: open_group(tt, half),
                     (PAIRS - 1) * QC * _PE_PER_ROW))
            attn_chunk(3, range(PAIRS), fillers, debt)
            while fillers:
                fillers.popleft()[0]()

            for tt, half in g3[2:]:
                close_group()
                open_group(tt, half)
            while open_g:
                close_group()

    nc.compile()
    return nc


def _make_runner(nc):
    """Reusable 8-core SPMD runner (jit built once)."""
    import jax
    from jax.sharding import Mesh, PartitionSpec
    from jax.experimental.shard_map import shard_map
    from concourse import bass2jax
    import concourse.mybir as mybir

    bass2jax.install_neuronx_cc_hook()
    partition_name = (nc.partition_id_tensor.name
                      if nc.partition_id_tensor else None)
    in_names, out_names, out_avals, zero_outs = [], [], [], []
    for alloc in nc.m.functions[0].allocations:
        if not isinstance(alloc, mybir.MemoryLocationSet):
            continue
        name = alloc.memorylocations[0].name
        if alloc.kind == "ExternalInput":
            if name != partition_name:
                in_names.append(name)
        elif alloc.kind == "ExternalOutput":
            shape = tuple(alloc.tensor_shape)
            dtype = mybir.dt.np(alloc.dtype)
            out_names.append(name)
            out_avals.append(jax.core.ShapedArray(shape, dtype))
            zero_outs.append(np.zeros(shape, dtype))
    n_params = len(in_names)
    n_outs = len(out_avals)
    all_in = list(in_names) + list(out_names)
    if partition_name is not None:
        all_in.append(partition_name)

    def _body(*args):
        operands = list(args)
        if partition_name is not None:
            operands.append(bass2jax.partition_id_tensor())
        outs = bass2jax._bass_exec_p.bind(
            *operands,
            out_avals=tuple(out_avals),
            in_names=tuple(all_in),
            out_names=tuple(out_names),
            lowering_input_output_aliases=(),
            sim_require_finite=True,
            sim_require_nnan=True,
            nc=nc,
        )
        return tuple(outs)

    devices = jax.devices()[:N_CORES]
    mesh = Mesh(np.asarray(devices), ("core",))
    in_specs = (PartitionSpec("core"),) * (n_params + n_outs)
    out_specs = (PartitionSpec("core"),) * n_outs
    donate = tuple(range(n_params, n_params + n_outs))
    sharded = jax.jit(
        shard_map(_body, mesh=mesh, in_specs=in_specs, out_specs=out_specs,
                  check_rep=False),
        donate_argnums=donate, keep_unused=True)

    def run(in_maps):
        per_core = [[np.asarray(m[k]) for k in in_names] for m in in_maps]
        concat_in = [
            np.concatenate([per_core[c][i] for c in range(N_CORES)], axis=0)
            for i in range(n_params)]
        concat_zeros = [
            np.zeros((N_CORES * z.shape[0], *z.shape[1:]), z.dtype)
            for z in zero_outs]
        outs = sharded(*concat_in, *concat_zeros)
        jax.block_until_ready(outs)
        return [
            {name: np.asarray(outs[i]).reshape(N_CORES, *out_avals[i].shape)[c]
             for i, name in enumerate(out_names)}
            for c in range(N_CORES)]

    return run


def kernel(x, w_qkv, b_qkv, w_proj, b_proj):
    x = np.asarray(x, dtype=np.float32)
    w_qkv = np.asarray(w_qkv, dtype=np.float32)
    b_qkv = np.asarray(b_qkv, dtype=np.float32)
    w_proj = np.asarray(w_proj, dtype=np.float32)
    b_proj = np.asarray(b_proj, dtype=np.float32)

    w_q, w_k, w_v = w_qkv[0:1024], w_qkv[1024:2048], w_qkv[2048:3072]
    b_q, b_k, b_v = b_qkv[0:1024], b_qkv[1024:2048], b_qkv[2048:3072]
    has_qk_bias = bool(np.any(b_q) or np.any(b_k))

    key = ("runner", has_qk_bias)
    if key not in _RUNNER_CACHE:
        nc = _build(has_qk_bias)
        _RUNNER_CACHE[key] = _make_runner(nc)
    run = _RUNNER_CACHE[key]

    # causal transition-block mask: tri[k, i] = 1.0 iff k <= i
    kk = np.arange(P)
    tri = (kk[:, None] <= kk[None, :]).astype(np.float16)
    idn = np.eye(P, dtype=np.float32)

    in_maps = []
    for core in range(N_CORES):
        b, g = divmod(core, 2)
        xT_c = x[b].T.astype(np.float16)  # [1024, 2048]
        if has_qk_bias:
            xT_c = np.concatenate(
                [xT_c, np.ones((1, T), np.float16)], axis=0)
        KD = D_MODEL + (1 if has_qk_bias else 0)
        wa_c = np.empty((KD, W_COLS), np.float32)
        wp_c = np.empty((512, 1024), np.float32)
        for p in range(PAIRS):
            hA = 8 * g + 2 * p
            hB = hA + 1
            cols = p * 256
            wa_c[:D_MODEL, cols + 0:cols + 64] = w_q[hA * 64:(hA + 1) * 64].T
            wa_c[:D_MODEL, cols + 64:cols + 128] = w_q[hB * 64:(hB + 1) * 64].T
            wa_c[:D_MODEL, cols + 128:cols + 192] = w_k[hA * 64:(hA + 1) * 64].T
            wa_c[:D_MODEL, cols + 192:cols + 256] = w_k[hB * 64:(hB + 1) * 64].T
            if has_qk_bias:
                wa_c[D_MODEL, cols + 0:cols + 64] = b_q[hA * 64:(hA + 1) * 64]
                wa_c[D_MODEL, cols + 64:cols + 128] = b_q[hB * 64:(hB + 1) * 64]
                wa_c[D_MODEL, cols + 128:cols + 192] = b_k[hA * 64:(hA + 1) * 64]
                wa_c[D_MODEL, cols + 192:cols + 256] = b_k[hB * 64:(hB + 1) * 64]
            # wp rows pair-major: [hA dims 0..63 | hB dims 64..127]
            wp_c[p * 128:p * 128 + 64, :] = w_proj.T[hA * 64:(hA + 1) * 64, :]
            wp_c[p * 128 + 64:p * 128 + 128, :] = \
                w_proj.T[hB * 64:(hB + 1) * 64, :]
        # v columns, head-major for the group
        for lh in range(8):
            head = 8 * g + lh
            wa_c[:D_MODEL, 1024 + lh * 64:1024 + (lh + 1) * 64] = \
                w_v[head * 64:(head + 1) * 64].T
            if has_qk_bias:
                wa_c[D_MODEL, 1024 + lh * 64:1024 + (lh + 1) * 64] = \
                    b_v[head * 64:(head + 1) * 64]
        in_maps.append({
            "xT": xT_c,
            "wa": wa_c.astype(np.float16),
            "wp": wp_c.astype(np.float16),
            "tri": tri,
            "idn": idn,
        })

    results = run(in_maps)

    # partial-sum unshard: the two head-group cores of each batch each
    # produced out_partial[T, 1024]; add them.
    out = np.empty((B, T, D_MODEL), dtype=np.float32)
    for b in range(B):
        out[b] = (results[2 * b]["out"].astype(np.float32)
                  + results[2 * b + 1]["out"].astype(np.float32))

    # exact host-side bias folds (v-bias rides softmax row-sums == 1 and is
    # on-device in the qk-bias build; proj bias is additive)
    if np.any(b_v) and not has_qk_bias:
        out += (b_v @ w_proj.T)[None, None, :]
    if np.any(b_proj):
        out += b_proj[None, None, :]
    return out
